# revision 1
# baseline (speedup 1.0000x reference)
"""Multi-head attention (B=4, S=2048, E=768, H=8, D=96) on 8 Trainium2 cores.

Sharding: core c -> (batch b = c//2, head-group hg = c%2 of 4 heads).
Each core computes Q/K/V projections for its 4 heads over the full sequence
of its batch, full attention for those heads, and a partial output
projection (row-split Wo).  The two cores of a batch produce partial
outputs that are summed on the host during unsharding (tensor-parallel
reduce).

On-chip layout notes:
  - All matmul operands are bf16 (1 cycle/row on PE; fp32 would be 4x;
    fp8 DoubleRow would halve PE time but its ~3% per-element quantization
    noise lands ~1:1 in the output - the softmax average does not shrink
    relative error for zero-mean V - and busts the 2e-2 tolerance).
  - head_dim 96 is zero-padded to 128 (host pads Wq/Wk columns), so every
    matmul has K=128 contraction and 128-column stationary operands (FWL).
  - Attention scores are computed transposed, S^T[k, q] = K^T.T @ Q^T,
    so softmax normalization is a partition reduction; we get the sums for
    free by augmenting V with a ones column (row 96 of the O^T accumulator
    is then sum_k exp(S)).
  - exp runs on the scalar engine straight out of PSUM ([128,1024] over a
    pair of key tiles) with the 1/sqrt(d) scale folded into the
    activation's scale parameter.  PE is the bottleneck engine (~88%
    busy); ACT has slack, so it also absorbs some PSUM evictions.
  - Per-(head, q-chunk) normalization: sums row -> DRAM -> broadcast-DMA
    to 96 partitions -> fast reciprocal on DVE -> tensor_tensor mults.
    The two sums DMAs ride the Pool/SWDGE queue so they never head-of-line
    block the SP queue that carries input loads and output stores.  For
    the LAST head the roundtrip (~6 us) would stall the dependent
    output-projection chunks, so there the sums row is broadcast across
    96 partitions with a K=1 PE matmul against a ones column (~0.3 us),
    and the normalization multiplies are sliced per 128-column
    output-projection tile so each tile unblocks as early as possible.
  - Inputs are host-packed so each operand is ONE [128, n] DMA (the SP
    sequencer spends ~650 ns per DMA issue; 47 small input DMAs would
    serialize into a ~26 us head).  x is packed seq-block-major so the
    first K/Q chunk needs only wk + 0.75MB; biases load early because
    they gate the K/Q psum evictions; head 0's K/Q weight columns load
    before the other heads'.
  - PE stream order: head 0's K/Q projections first (they gate the ACT
    exp stream), V projection chunks interleaved per x block, then the
    attention pipeline; K/Q chunks of head h+1 are interleaved into head
    h's attention stream.  Output-projection chunks can only run
    during/after the last head (they need all four heads' softmax
    normalizations), but head 3's attnT rows live solely in packed tile
    2 - so each chunk splits into a t0/t1 partial (independent of the
    newest normalization, injected at the bare front pairs of the
    following q-chunk's stream) and a t2 finish (accumulation close +
    evictions + split stores) injected from pair 3.  The final q-chunk's
    four chunks run the same two-phase way in the tail, borrowing the
    then-idle attention psum rings so the partials overlap the last
    normalization chain.
"""

import os
import sys

sys.path.insert(0, "/opt/trn_rl_repo")

import numpy as np
import ml_dtypes

import concourse.bacc as bacc
import concourse.bass as bass
import concourse.tile as tile
from concourse import mybir
from concourse.bass_utils import run_bass_kernel_spmd

BF16 = ml_dtypes.bfloat16

EMB = 768
HEADS = 8
HD = 96          # true head dim
HDP = 128        # padded head dim
SEQ = 2048
B = 4
NCORES = 8
HPC = 4          # heads per core
SCALING = HD ** -0.5
QC = 512         # query chunk per attention inner loop
NQC = SEQ // QC
NKT = SEQ // 128  # 16 key tiles
NPAIR = NKT // 2
NE = EMB // 128   # 6 e_in tiles

_NC_CACHE = {}
LAST_RESULT = None  # BassKernelResults of the most recent run (for test.py)


def _build_nc():
    f32 = mybir.dt.float32
    bf = mybir.dt.bfloat16

    nc = bacc.Bacc(trn_type="TRN2", target_bir_lowering=False, debug=False,
                   num_devices=NCORES)

    # All operands host-packed into [128, n] so each loads as ONE DMA.
    xtp = nc.dram_tensor("xtp", [128, NE * SEQ], bf, kind="ExternalInput").ap()
    # K/Q weights split: head 0's columns load first (0.19MB) so head 0's
    # projections - which gate the whole pipeline - start ~5 us earlier
    wqp0 = nc.dram_tensor("wqp0", [128, NE * HDP], bf,
                          kind="ExternalInput").ap()
    wqpr = nc.dram_tensor("wqpr", [128, NE * 3 * HDP], bf,
                          kind="ExternalInput").ap()
    wkp0 = nc.dram_tensor("wkp0", [128, NE * HDP], bf,
                          kind="ExternalInput").ap()
    wkpr = nc.dram_tensor("wkpr", [128, NE * 3 * HDP], bf,
                          kind="ExternalInput").ap()
    wvp = nc.dram_tensor("wvp", [128, NE * HPC * HD], bf,
                         kind="ExternalInput").ap()
    wop = nc.dram_tensor("wop", [128, 3 * EMB], bf, kind="ExternalInput").ap()
    bqp = nc.dram_tensor("bqp", [128, HPC], f32, kind="ExternalInput").ap()
    bkp = nc.dram_tensor("bkp", [128, HPC], f32, kind="ExternalInput").ap()
    outp = nc.dram_tensor("outp", [SEQ, EMB], f32, kind="ExternalOutput").ap()
    sums_dram = nc.dram_tensor("sums_scratch", [HPC * NQC, QC], f32).ap()

    with tile.TileContext(nc) as tc:
        with (
            tc.tile_pool(name="const", bufs=1) as constp,
            tc.tile_pool(name="big", bufs=1) as bigp,
            tc.tile_pool(name="expp", bufs=6) as expp,
            tc.tile_pool(name="rbp", bufs=4) as rbp,
            tc.tile_pool(name="outsb", bufs=12) as outsb,
            tc.tile_pool(name="ps_proj", bufs=2, space="PSUM") as ps_proj,
            tc.tile_pool(name="ps_o", bufs=2, space="PSUM") as ps_o,
            tc.tile_pool(name="ps_pair", bufs=2, space="PSUM") as ps_pair,
        ):
            # ---- loads. x is packed seq-block-major ([128, 6e x 512] per
            # 512-sequence block) so the first K/Q chunk only needs wk + one
            # 0.75MB block; wk + block 0/1 load first, k/q chunks of head 0
            # then pipeline behind the remaining block DMAs. ----
            XB = NE * 512  # 3072 cols per seq block
            wk0_sb = constp.tile([128, NE * HDP], bf, name="wk0_sb")
            nc.sync.dma_start(out=wk0_sb, in_=wkp0)
            xtb = [bigp.tile([128, XB], bf, name=f"xtb{n}") for n in range(4)]
            nc.sync.dma_start(out=xtb[0][:, 0:XB // 2], in_=xtp[:, 0:XB // 2])
            nc.sync.dma_start(out=xtb[0][:, XB // 2:XB],
                              in_=xtp[:, XB // 2:XB])
            # biases next: tiny, but they gate the K/Q psum evictions
            bq_sb = constp.tile([128, HPC], f32, name="bq_sb")
            nc.sync.dma_start(out=bq_sb, in_=bqp)
            bk_sb = constp.tile([128, HPC], f32, name="bk_sb")
            nc.sync.dma_start(out=bk_sb, in_=bkp)
            wq0_sb = constp.tile([128, NE * HDP], bf, name="wq0_sb")
            nc.sync.dma_start(out=wq0_sb, in_=wqp0)
            ones96 = constp.tile([1, HD], bf, name="ones96")
            nc.gpsimd.memset(ones96, 1.0)
            wv_sb = constp.tile([128, NE * HPC * HD], bf, name="wv_sb")
            nc.sync.dma_start(out=wv_sb, in_=wvp)
            nc.sync.dma_start(out=xtb[1], in_=xtp[:, XB:2 * XB])
            wkr_sb = constp.tile([128, NE * 3 * HDP], bf, name="wkr_sb")
            nc.sync.dma_start(out=wkr_sb, in_=wkpr)
            wqr_sb = constp.tile([128, NE * 3 * HDP], bf, name="wqr_sb")
            nc.sync.dma_start(out=wqr_sb, in_=wqpr)
            nc.sync.dma_start(out=xtb[2], in_=xtp[:, 2 * XB:3 * XB])
            nc.sync.dma_start(out=xtb[3], in_=xtp[:, 3 * XB:4 * XB])
            wo_sb = constp.tile([128, 3 * EMB], bf, name="wo_sb")
            nc.sync.dma_start(out=wo_sb, in_=wop)

            def wk_eh(e, h):
                if h == 0:
                    return wk0_sb[:, e * HDP:(e + 1) * HDP]
                return wkr_sb[:, (e * 3 + h - 1) * HDP:(e * 3 + h) * HDP]

            def wq_eh(e, h):
                if h == 0:
                    return wq0_sb[:, e * HDP:(e + 1) * HDP]
                return wqr_sb[:, (e * 3 + h - 1) * HDP:(e * 3 + h) * HDP]

            def wv_e(e):
                return wv_sb[:, e * HPC * HD:(e + 1) * HPC * HD]

            def wo_t(t_):
                return wo_sb[:, t_ * EMB:(t_ + 1) * EMB]

            # ---- persistent intermediates ----
            vaug = []
            for kt in range(NKT):
                t = bigp.tile([128, HPC * HDP], bf, name=f"vaug{kt}")
                nc.gpsimd.memset(t, 0.0)
                ones_cols = t.rearrange("p (h c) -> p h c", h=HPC)[:, :, HD:HD + 1]
                nc.gpsimd.memset(ones_cols, 1.0)
                vaug.append(t)
            qT = [bigp.tile([128, SEQ], bf, name=f"qT{h}") for h in range(HPC)]
            kT = [bigp.tile([128, SEQ], bf, name=f"kT{h}") for h in range(HPC)]
            # packed attention output, [384 rows = 3 tiles x 128, seq]; every
            # row is written by the normalization TTs, so no memset needed
            attnT = [bigp.tile([128, SEQ], bf, name=f"attnT{t_}")
                     for t_ in range(3)]

            def head_blocks(h):
                """32-row blocks mapping head h's 96 rows into packed attnT
                (all partition starts/spans quadrant-legal)."""
                out = []
                for b_ in range(HD // 32):
                    g = HD * h + 32 * b_
                    out.append((g // 128, g % 128, 32 * b_))
                return out

            f32_ = f32

            # ---- projection emit helpers ----
            def emit_v_chunk(kt):
                psv = ps_proj.tile([128, 512], f32_, tag="ps",
                                   name=f"psv{kt}")
                blk, off = divmod(kt, 4)
                for e in range(NE):
                    nc.tensor.matmul(psv[:, 0:HPC * HD],
                                     lhsT=xtb[blk][:, e * 512 + off * 128:
                                                   e * 512 + off * 128 + 128],
                                     rhs=wv_e(e),
                                     start=(e == 0), stop=(e == NE - 1))
                for hh in range(HPC):
                    nc.vector.tensor_copy(
                        vaug[kt][:, hh * HDP:hh * HDP + HD],
                        psv[:, hh * HD:(hh + 1) * HD])

            kq_ps = {}

            def emit_kq_part(h, n, which, part):
                """One half of a K/Q chunk (3 of 6 e-matmuls).  Thunking
                chunks at this granularity keeps the per-pair PE load above
                the ACT exp cadence, so the S stream never outruns ACT into
                the 2-slot pss ring."""
                key = (h, n, which)
                w_eh, dst, b_sb = ((wk_eh, kT, bk_sb) if which == "k"
                                   else (wq_eh, qT, bq_sb))
                if part == 0:
                    kq_ps[key] = ps_proj.tile([128, 512], f32_, tag="ps",
                                              name=f"ps{which}{h}_{n}")
                ps = kq_ps[key]
                for e in (3 * part, 3 * part + 1, 3 * part + 2):
                    nc.tensor.matmul(ps,
                                     lhsT=w_eh(e, h),
                                     rhs=xtb[n][:, e * 512:(e + 1) * 512],
                                     start=(e == 0), stop=(e == NE - 1))
                if part == 1:
                    nsl = slice(n * 512, (n + 1) * 512)
                    nc.vector.tensor_scalar_add(dst[h][:, nsl],
                                                kq_ps.pop(key),
                                                b_sb[:, h:h + 1])

            def emit_kq_chunk(h, n, which):
                for part in range(2):
                    emit_kq_part(h, n, which, part)

            def kq_chunks(h):
                for n in range(4):
                    yield ("k", h, n)
                    yield ("q", h, n)

            # ---- output projection chunk (one 128-row q tile) ----
            # Split across two 1-bank psums so it can borrow ps_proj slots;
            # PSUM->SBUF copies go on DVE (ACT is busy with exp here).
            # Tail out-proj chunks, two-phase: head 3's rows live only in
            # packed attnT tile 2, so the t0/t1 matmuls are independent of
            # the final normalization and run DURING its chain; only the t2
            # matmul (accumulation close) waits for the per-qm norm slice.
            # Psums borrow the attention rings (free by then).
            tail_ps = {}

            def emit_out_partial(qm, windowed=False):
                qsl = slice(qm * 128, (qm + 1) * 128)
                if windowed:
                    psA = ps_proj.tile([128, 512], f32_, tag="ps",
                                       name=f"poA{qm}")
                    psB = ps_proj.tile([128, 512], f32_, tag="ps",
                                       name=f"poB{qm}")
                else:
                    psA = ps_pair.tile([128, 512], f32_, tag="pss",
                                       name=f"poA{qm}")
                    psB = ps_o.tile([128, 512], f32_, tag="pso",
                                    name=f"poB{qm}")
                tail_ps[qm] = (psA, psB)
                for t in range(2):
                    nc.tensor.matmul(psA,
                                     lhsT=attnT[t][:, qsl],
                                     rhs=wo_t(t)[:, 0:512],
                                     start=(t == 0), stop=False)
                    nc.tensor.matmul(psB[:, 0:256],
                                     lhsT=attnT[t][:, qsl],
                                     rhs=wo_t(t)[:, 512:768],
                                     start=(t == 0), stop=False)

            def emit_out_finish(qm, windowed=False):
                qsl = slice(qm * 128, (qm + 1) * 128)
                psA, psB = tail_ps.pop(qm)
                nc.tensor.matmul(psA, lhsT=attnT[2][:, qsl],
                                 rhs=wo_t(2)[:, 0:512],
                                 start=False, stop=True)
                nc.tensor.matmul(psB[:, 0:256], lhsT=attnT[2][:, qsl],
                                 rhs=wo_t(2)[:, 512:768],
                                 start=False, stop=True)
                out_sb = outsb.tile([128, EMB], f32_, tag="osb",
                                    name=f"osb{qm}")
                # psA eviction on ACT; the store is split so each half
                # leaves right after its eviction
                nc.scalar.activation(out_sb[:, 0:512], psA,
                                     mybir.ActivationFunctionType.Copy)
                nc.sync.dma_start(out=outp[qm * 128:(qm + 1) * 128, 0:512],
                                  in_=out_sb[:, 0:512])
                nc.vector.tensor_copy(out_sb[:, 512:768], psB[:, 0:256])
                nc.sync.dma_start(out=outp[qm * 128:(qm + 1) * 128, 512:768],
                                  in_=out_sb[:, 512:768])

            # ---- attention emit (with interleaved PE filler work) ----
            def emit_attention(h, thunks_for_qc):
                """thunks_for_qc(qc) -> list of emit callables injected into
                the PE stream spread across this q-chunk's pairs."""
                hsl = slice(h * HDP, (h + 1) * HDP)
                for qc in range(NQC):
                    thunks = list(thunks_for_qc(qc))
                    inject_at = {}
                    if h == HPC - 1:
                        # P0 fills the bare front; finishes (which wait on
                        # the previous q-chunk's norm, landing ~pair 2-3)
                        # follow; the 2-slot psum ring forces P_k after
                        # F_{k-1}
                        pos = [0, 3, 4, 5, 6, 6, 7, 7]
                        for i, t in enumerate(thunks):
                            inject_at.setdefault(
                                pos[i] if i < len(pos) else NPAIR - 1,
                                []).append(t)
                    else:
                        step = max(NPAIR // max(len(thunks), 1), 1)
                        for i, t in enumerate(thunks):
                            inject_at.setdefault(
                                min(1 + i * step, NPAIR - 1), []).append(t)
                    qsl = slice(qc * QC, (qc + 1) * QC)
                    idx = h * NQC + qc
                    pso = ps_o.tile([128, QC], f32_, tag="pso",
                                    name=f"pso{idx}")
                    eps = []

                    def emit_ss(p):
                        pss = ps_pair.tile([128, 1024], f32_, tag="pss",
                                           name=f"pss{idx}_{p}")
                        for j in range(2):
                            nc.tensor.matmul(
                                pss[:, j * 512:(j + 1) * 512],
                                lhsT=kT[h][:, (2 * p + j) * 128:
                                           (2 * p + j + 1) * 128],
                                rhs=qT[h][:, qsl],
                                start=True, stop=True)
                        ep = expp.tile([128, 1024], bf, tag="exp",
                                       name=f"exp{idx}_{p}")
                        nc.scalar.activation(ep, pss,
                                             mybir.ActivationFunctionType.Exp,
                                             scale=SCALING)
                        eps.append(ep)

                    def emit_o(p):
                        for j in range(2):
                            kt = 2 * p + j
                            nc.tensor.matmul(
                                pso,
                                lhsT=vaug[kt][:, hsl],
                                rhs=eps[p][:, j * 512:(j + 1) * 512],
                                start=(kt == 0), stop=(kt == NKT - 1))

                    for p in range(NPAIR):
                        emit_ss(p)
                        for t in inject_at.get(p, ()):
                            t()
                        if p >= 1:
                            emit_o(p - 1)
                    emit_o(NPAIR - 1)

                    # bf16 sums for head 3: feeds the K=1 broadcast matmul
                    # (bf16 streams at 1 cycle/row vs fp32's 4); the ~0.4%
                    # rounding on one normalization factor is negligible
                    sums_sb = rbp.tile([1, QC], bf if h == HPC - 1 else f32_,
                                       tag="sums", name=f"sums{idx}")
                    nc.vector.tensor_copy(sums_sb, pso[HD:HD + 1, :])
                    rb2 = rbp.tile([HD, QC], f32_, tag="rb2",
                                   name=f"rb2{idx}")
                    if h == HPC - 1:
                        # latency-critical (out-proj thunks / tail wait on
                        # it): broadcast the sums row over 96 partitions with
                        # a K=1 matmul instead of the ~6us DRAM store +
                        # broadcast roundtrip
                        rbps = ps_proj.tile([HD, QC], f32_, tag="ps",
                                            name=f"rbps{idx}")
                        nc.tensor.matmul(rbps, lhsT=ones96, rhs=sums_sb,
                                         start=True, stop=True)
                        nc.vector.reciprocal_approx_fast(out=rb2, in_=rbps)
                    else:
                        # sums roundtrip on the Pool/SWDGE queue: keeps these
                        # dependent DMAs from head-of-line blocking SP
                        nc.gpsimd.dma_start(out=sums_dram[idx:idx + 1, :],
                                            in_=sums_sb)
                        rb = rbp.tile([HD, QC], f32_, tag="rb",
                                      name=f"rb{idx}")
                        nc.gpsimd.dma_start(
                            out=rb,
                            in_=sums_dram[idx:idx + 1, :]
                            .to_broadcast([HD, QC]))
                        nc.vector.reciprocal_approx_fast(out=rb2, in_=rb)
                    if h == HPC - 1:
                        # last head: normalize per out-proj 128-col slice so
                        # dependent out-proj chunks start one slice at a
                        # time instead of waiting for the full 512 columns
                        for qmi in range(4):
                            csl = slice(qmi * 128, (qmi + 1) * 128)
                            osl = slice(qc * QC + qmi * 128,
                                        qc * QC + (qmi + 1) * 128)
                            for t_, off, src in head_blocks(h):
                                nc.vector.tensor_mul(
                                    out=attnT[t_][off:off + 32, osl],
                                    in0=pso[src:src + 32, csl],
                                    in1=rb2[src:src + 32, csl])
                    else:
                        for t_, off, src in head_blocks(h):
                            nc.vector.tensor_mul(
                                out=attnT[t_][off:off + 32, qsl],
                                in0=pso[src:src + 32, :],
                                in1=rb2[src:src + 32, :])

            # ---- emission schedule ----
            # consume each x seq-block as its DMA lands: head 0's k/q chunks
            # for block n, then the V chunks of block n
            for n in range(4):
                emit_kq_chunk(0, n, "k")
                emit_kq_chunk(0, n, "q")
                for kt in range(4 * n, 4 * n + 4):
                    emit_v_chunk(kt)

            def kq_thunks(hnext):
                def f(qc):
                    # 2 chunks per q-chunk: 8 chunks over 4 qcs
                    items = list(kq_chunks(hnext))[2 * qc:2 * qc + 2]
                    return [lambda it=it: emit_kq_chunk(it[1], it[2], it[0])
                            for it in items]
                return f

            def out_thunks(qc):
                # during h3's q-chunk qc, emit out-proj rows of q-chunk
                # qc-1 two-phase: t0/t1 partials need only heads 0-2 (done
                # long ago) so they fill the bare front pairs before this
                # window's normalization lands; t2 finishes follow.  In the
                # LAST window only 3 chunks are thunked - the 4th would sit
                # at pair 7 and delay the last O matmul that gates the
                # final normalization; it runs right after the block
                # instead, overlapping the norm chain.
                if qc == 0:
                    return []
                out = []
                for qm in range(4 * (qc - 1), 4 * qc):
                    out.append(lambda qm=qm: emit_out_partial(qm, True))
                    out.append(lambda qm=qm: emit_out_finish(qm, True))
                return out

            for h in range(HPC - 1):
                emit_attention(h, kq_thunks(h + 1))
            emit_attention(HPC - 1, out_thunks)
            q0 = 4 * (NQC - 1)
            emit_out_partial(q0)
            emit_out_partial(q0 + 1)
            emit_out_finish(q0)
            emit_out_partial(q0 + 2)
            emit_out_finish(q0 + 1)
            emit_out_partial(q0 + 3)
            emit_out_finish(q0 + 2)
            emit_out_finish(q0 + 3)

    nc.compile()
    return nc


def _get_nc():
    if "nc" not in _NC_CACHE:
        _NC_CACHE["nc"] = _build_nc()
    return _NC_CACHE["nc"]


def _etile_pack(wT):
    """[768, n] (e on rows) -> [128, 6*n] bf16: e-tiles as column blocks so
    the whole operand loads as ONE [128, n] DMA."""
    n = wT.shape[1]
    a = wT.reshape(NE, 128, n).transpose(1, 0, 2)
    return np.ascontiguousarray(a.reshape(128, NE * n)).astype(BF16)


def _x_block_pack(x_b):
    """[2048, 768] x -> [128, 4 * 6 * 512] bf16, seq-block-major: block n
    holds e-tiles of sequence rows n*512..(n+1)*512 as column slabs."""
    a = x_b.reshape(4, 512, NE, 128)          # n, s, e, p
    a = a.transpose(3, 0, 2, 1)               # p, n, e, s
    return np.ascontiguousarray(a.reshape(128, 4 * NE * 512)).astype(BF16)


def _pad_headsT(w_rows):
    """[384, 768] head rows -> zero-pad head dim 96->128 -> transpose -> [768, 512]."""
    p = np.zeros((HPC * HDP, EMB), np.float32)
    p.reshape(HPC, HDP, EMB)[:, :HD] = w_rows.reshape(HPC, HD, EMB)
    return np.ascontiguousarray(p.T)


def _pad_bias(b_rows):
    """[384] head bias -> [128, HPC] padded/transposed for per-partition add."""
    p = np.zeros((HPC, HDP), np.float32)
    p[:, :HD] = b_rows.reshape(HPC, HD)
    return np.ascontiguousarray(p.T)


def kernel(x, Wq, bq, Wk, bk, Wv, bv, Wo, bo):
    x = np.asarray(x, np.float32)
    Wq, bq = np.asarray(Wq, np.float32), np.asarray(bq, np.float32)
    Wk, bk = np.asarray(Wk, np.float32), np.asarray(bk, np.float32)
    Wv, bv = np.asarray(Wv, np.float32), np.asarray(bv, np.float32)
    Wo, bo = np.asarray(Wo, np.float32), np.asarray(bo, np.float32)

    nc = _get_nc()

    in_maps = []
    for c in range(NCORES):
        b, hg = divmod(c, 2)
        hs = slice(hg * HPC * HD, (hg + 1) * HPC * HD)
        woT = Wo[:, hs].T  # [384, 768]
        wo_pack = np.ascontiguousarray(
            woT.reshape(3, 128, EMB).transpose(1, 0, 2).reshape(128, 3 * EMB))
        wq_et = _etile_pack(_pad_headsT(Wq[hs])).reshape(128, NE, HPC, HDP)
        wk_et = _etile_pack(_pad_headsT(Wk[hs])).reshape(128, NE, HPC, HDP)
        in_maps.append({
            "xtp": _x_block_pack(x[b]),
            "wqp0": np.ascontiguousarray(
                wq_et[:, :, 0].reshape(128, NE * HDP)),
            "wqpr": np.ascontiguousarray(
                wq_et[:, :, 1:].reshape(128, NE * 3 * HDP)),
            "wkp0": np.ascontiguousarray(
                wk_et[:, :, 0].reshape(128, NE * HDP)),
            "wkpr": np.ascontiguousarray(
                wk_et[:, :, 1:].reshape(128, NE * 3 * HDP)),
            "wvp": _etile_pack(np.ascontiguousarray(Wv[hs].T)),
            "wop": wo_pack.astype(BF16),
            "bqp": _pad_bias(bq[hs]),
            "bkp": _pad_bias(bk[hs]),
        })

    global LAST_RESULT
    trace = bool(int(os.environ.get("KERNEL_TRACE", "0")))
    tmpdir = os.environ.get("KERNEL_TRACE_DIR") or None
    res = run_bass_kernel_spmd(nc, in_maps, list(range(NCORES)), trace=trace,
                               tmpdir=tmpdir)
    LAST_RESULT = res

    out = np.empty((B, SEQ, EMB), np.float32)
    for b in range(B):
        out[b] = res.results[2 * b]["outp"] + res.results[2 * b + 1]["outp"]
    # bv enters each head's output additively (sum of softmax weights is 1),
    # and bo is a plain add: both fold into one constant vector.
    out += Wo @ bv + bo
    return out



# revision 51
# speedup vs baseline: 1.0684x; 1.0684x over previous
"""Multi-head attention (B=4, S=2048, E=768, H=8, D=96) on 8 Trainium2 cores.

Sharding: core c -> (batch b = c//2, head-group hg = c%2 of 4 heads).
Each core computes Q/K/V projections for its 4 heads over the full sequence
of its batch, full attention for those heads, and a partial output
projection (row-split Wo).  The two cores of a batch produce partial
outputs that are summed on the host during unsharding (tensor-parallel
reduce).

On-chip layout notes:
  - All matmul operands are bf16 (1 cycle/row on PE; fp32 would be 4x;
    fp8 DoubleRow would halve PE time but its ~3% per-element quantization
    noise lands ~1:1 in the output - the softmax average does not shrink
    relative error for zero-mean V - and busts the 2e-2 tolerance).
  - head_dim 96 is zero-padded to 128 (host pads Wq/Wk columns), so every
    matmul has K=128 contraction and 128-column stationary operands (FWL).
  - Attention scores are computed transposed, S^T[k, q] = K^T.T @ Q^T,
    so softmax normalization is a partition reduction; we get the sums for
    free by augmenting V with a ones column (row 96 of the O^T accumulator
    is then sum_k exp(S)).
  - exp runs on the scalar engine straight out of PSUM ([128,1024] over a
    pair of key tiles) with the 1/sqrt(d) scale folded into the
    activation's scale parameter.  PE is the bottleneck engine (~88%
    busy); ACT has slack, so it also absorbs some PSUM evictions.
  - Per-(head, q-chunk) normalization: sums row -> DRAM -> broadcast-DMA
    to 96 partitions -> fast reciprocal on DVE -> tensor_tensor mults.
    The two sums DMAs ride the Pool/SWDGE queue so they never head-of-line
    block the SP queue that carries input loads and output stores.  For
    the LAST head the roundtrip (~6 us) would stall the dependent
    output-projection chunks, so there the sums row is broadcast across
    96 partitions with a K=1 PE matmul against a ones column (~0.3 us),
    and the normalization multiplies are sliced per 128-column
    output-projection tile so each tile unblocks as early as possible.
  - Inputs are host-packed so each operand is ONE [128, n] DMA (the SP
    sequencer spends ~650 ns per DMA issue; 47 small input DMAs would
    serialize into a ~26 us head).  x is packed seq-block-major so the
    first K/Q chunk needs only wk + 0.75MB; biases load early because
    they gate the K/Q psum evictions; head 0's K/Q weight columns load
    before the other heads'.
  - PE stream order: head 0's K/Q projections first (they gate the ACT
    exp stream), V projection chunks interleaved per x block, then the
    attention pipeline; K/Q chunks of head h+1 are interleaved into head
    h's attention stream.  Output-projection chunks can only run
    during/after the last head (they need all four heads' softmax
    normalizations), but head 3's attnT rows live solely in packed tile
    2 - so each chunk splits into a t0/t1 partial (independent of the
    newest normalization, injected at the bare front pairs of the
    following q-chunk's stream) and a t2 finish (accumulation close +
    evictions + split stores) injected from pair 3.  The final q-chunk's
    four chunks run the same two-phase way in the tail, borrowing the
    then-idle attention psum rings so the partials overlap the last
    normalization chain.
"""

import os
import sys

sys.path.insert(0, "/opt/trn_rl_repo")

import numpy as np
import ml_dtypes

import concourse.bacc as bacc
import concourse.bass as bass
import concourse.tile as tile
from concourse import mybir
from concourse.bass_utils import run_bass_kernel_spmd

BF16 = ml_dtypes.bfloat16


def _pool_activation(nc, out, in_, func, scale=1.0):
    """Emit an InstActivation on the Pool/GPSIMD engine (bass only defines
    activation() on the scalar engine; the instruction itself is engine-
    tagged by the emitting queue, and the gpsimd DSP implements it)."""
    return bass.BassScalarEngine.activation(
        nc.gpsimd, out, in_, func, scale=scale)

EMB = 768
HEADS = 8
HD = 96          # true head dim
HDP = 128        # padded head dim
SEQ = 2048
B = 4
NCORES = 8
HPC = 4          # heads per core
SCALING = HD ** -0.5
QC = 512         # query chunk per attention inner loop
NQC = SEQ // QC
NKT = SEQ // 128  # 16 key tiles
NPAIR = NKT // 2
NE = EMB // 128   # 6 e_in tiles

_NC_CACHE = {}
LAST_RESULT = None  # BassKernelResults of the most recent run (for test.py)


def _build_nc():
    f32 = mybir.dt.float32
    bf = mybir.dt.bfloat16

    nc = bacc.Bacc(trn_type="TRN2", target_bir_lowering=False, debug=False,
                   num_devices=NCORES)

    # All operands host-packed into [128, n] so each loads as ONE DMA.
    xtp = nc.dram_tensor("xtp", [128, NE * SEQ], bf, kind="ExternalInput").ap()
    # K/Q weights split: head 0's columns load first (0.19MB) so head 0's
    # projections - which gate the whole pipeline - start ~5 us earlier
    wqp0 = nc.dram_tensor("wqp0", [128, NE * HDP], bf,
                          kind="ExternalInput").ap()
    wqpr = nc.dram_tensor("wqpr", [128, NE * 3 * HDP], bf,
                          kind="ExternalInput").ap()
    wkp0 = nc.dram_tensor("wkp0", [128, NE * HDP], bf,
                          kind="ExternalInput").ap()
    wkpr = nc.dram_tensor("wkpr", [128, NE * 3 * HDP], bf,
                          kind="ExternalInput").ap()
    wvp = nc.dram_tensor("wvp", [128, NE * HPC * HD], bf,
                         kind="ExternalInput").ap()
    wop = nc.dram_tensor("wop", [128, 3 * EMB], bf, kind="ExternalInput").ap()
    bqp = nc.dram_tensor("bqp", [128, HPC], f32, kind="ExternalInput").ap()
    bkp = nc.dram_tensor("bkp", [128, HPC], f32, kind="ExternalInput").ap()
    identp = nc.dram_tensor("identp", [128, 128], bf,
                            kind="ExternalInput").ap()
    outp = nc.dram_tensor("outp", [SEQ, EMB], f32, kind="ExternalOutput").ap()

    with tile.TileContext(nc) as tc:
        with (
            tc.tile_pool(name="const", bufs=1) as constp,
            tc.tile_pool(name="big", bufs=1) as bigp,
            tc.tile_pool(name="expp", bufs=6) as expp,
            tc.tile_pool(name="rbp", bufs=4) as rbp,
            tc.tile_pool(name="normp", bufs=1) as normp,
            tc.tile_pool(name="outsb", bufs=12) as outsb,
            tc.tile_pool(name="ps_proj", bufs=2, space="PSUM") as ps_proj,
            tc.tile_pool(name="ps_o", bufs=2, space="PSUM") as ps_o,
            tc.tile_pool(name="ps_pair", bufs=2, space="PSUM") as ps_pair,
        ):
            # ---- loads. x is packed seq-block-major ([128, 6e x 512] per
            # 512-sequence block) so the first K/Q chunk only needs wk + one
            # 0.75MB block; wk + block 0/1 load first, k/q chunks of head 0
            # then pipeline behind the remaining block DMAs. ----
            XB = NE * 512  # 3072 cols per seq block
            wk0_sb = constp.tile([128, NE * HDP], bf, name="wk0_sb")
            nc.sync.dma_start(out=wk0_sb, in_=wkp0)
            xtb = [bigp.tile([128, XB], bf, name=f"xtb{n}") for n in range(4)]
            nc.sync.dma_start(out=xtb[0][:, 0:XB // 2], in_=xtp[:, 0:XB // 2])
            nc.sync.dma_start(out=xtb[0][:, XB // 2:XB],
                              in_=xtp[:, XB // 2:XB])
            # biases next: tiny, but they gate the K/Q psum evictions
            bq_sb = constp.tile([128, HPC], f32, name="bq_sb")
            nc.sync.dma_start(out=bq_sb, in_=bqp)
            bk_sb = constp.tile([128, HPC], f32, name="bk_sb")
            nc.sync.dma_start(out=bk_sb, in_=bkp)
            wq0_sb = constp.tile([128, NE * HDP], bf, name="wq0_sb")
            nc.sync.dma_start(out=wq0_sb, in_=wqp0)
            wv_sb = constp.tile([128, NE * HPC * HD], bf, name="wv_sb")
            nc.sync.dma_start(out=wv_sb, in_=wvp)
            nc.sync.dma_start(out=xtb[1], in_=xtp[:, XB:2 * XB])
            wkr_sb = constp.tile([128, NE * 3 * HDP], bf, name="wkr_sb")
            nc.sync.dma_start(out=wkr_sb, in_=wkpr)
            wqr_sb = constp.tile([128, NE * 3 * HDP], bf, name="wqr_sb")
            nc.sync.dma_start(out=wqr_sb, in_=wqpr)
            nc.sync.dma_start(out=xtb[2], in_=xtp[:, 2 * XB:3 * XB])
            nc.sync.dma_start(out=xtb[3], in_=xtp[:, 3 * XB:4 * XB])
            wo_sb = constp.tile([128, 3 * EMB], bf, name="wo_sb")
            nc.sync.dma_start(out=wo_sb, in_=wop)
            # identity for the PE repack transposes (first needed in head
            # 3's phase, so it loads last)
            ident_sb = constp.tile([128, 128], bf, name="ident_sb")
            nc.sync.dma_start(out=ident_sb, in_=identp)

            def wk_eh(e, h):
                if h == 0:
                    return wk0_sb[:, e * HDP:(e + 1) * HDP]
                return wkr_sb[:, (e * 3 + h - 1) * HDP:(e * 3 + h) * HDP]

            def wq_eh(e, h):
                if h == 0:
                    return wq0_sb[:, e * HDP:(e + 1) * HDP]
                return wqr_sb[:, (e * 3 + h - 1) * HDP:(e * 3 + h) * HDP]

            def wv_e(e):
                return wv_sb[:, e * HPC * HD:(e + 1) * HPC * HD]

            def wo_t(t_):
                return wo_sb[:, t_ * EMB:(t_ + 1) * EMB]

            # ---- persistent intermediates ----
            # vaug: per key tile, 4 heads x (96 v-cols + a ones col).  The
            # ones col makes row^T @ vaug yield the softmax denominator in
            # the same accumulator (col 96 of each head group).  No pad
            # cols: vaug is the MOVING operand of the flipped O matmul, so
            # narrower means cheaper (97 vs 128 cycles).
            HDV = HD + 1
            vaug = []
            for kt in range(NKT):
                t = bigp.tile([128, HPC * HDV], bf, name=f"vaug{kt}")
                ones_cols = t.rearrange("p (h c) -> p h c",
                                        h=HPC)[:, :, HD:HD + 1]
                nc.gpsimd.memset(ones_cols, 1.0)
                vaug.append(t)
            qT = [bigp.tile([128, SEQ], bf, name=f"qT{h}") for h in range(HPC)]
            kT = [bigp.tile([128, SEQ], bf, name=f"kT{h}") for h in range(HPC)]
            # packed attention output, [384 rows = 3 tiles x 128, seq],
            # written by the h3-phase repack transposes
            attnT = [bigp.tile([128, SEQ], bf, name=f"attnT{t_}")
                     for t_ in range(3)]
            # normalized O in [q, head-dims] layout, per (q-chunk, q-tile):
            # written per head as its window completes, transposed into
            # attnT once all four heads are in
            normed = [[normp.tile([128, HPC * HD], bf, name=f"nm{qc}_{qt}")
                       for qt in range(4)] for qc in range(NQC)]

            f32_ = f32

            # ---- projection emit helpers ----
            def emit_v_chunk(kt):
                psv = ps_proj.tile([128, 512], f32_, tag="ps",
                                   name=f"psv{kt}")
                blk, off = divmod(kt, 4)
                for e in range(NE):
                    nc.tensor.matmul(psv[:, 0:HPC * HD],
                                     lhsT=xtb[blk][:, e * 512 + off * 128:
                                                   e * 512 + off * 128 + 128],
                                     rhs=wv_e(e),
                                     start=(e == 0), stop=(e == NE - 1))
                for hh in range(HPC):
                    nc.vector.tensor_copy(
                        vaug[kt][:, hh * HDV:hh * HDV + HD],
                        psv[:, hh * HD:(hh + 1) * HD])

            kq_ps = {}

            def emit_kq_part(h, n, which, part):
                """One half of a K/Q chunk (3 of 6 e-matmuls).  Thunking
                chunks at this granularity keeps the per-pair PE load above
                the ACT exp cadence, so the S stream never outruns ACT into
                the 2-slot pss ring."""
                key = (h, n, which)
                w_eh, dst, b_sb = ((wk_eh, kT, bk_sb) if which == "k"
                                   else (wq_eh, qT, bq_sb))
                if part == 0:
                    kq_ps[key] = ps_proj.tile([128, 512], f32_, tag="ps",
                                              name=f"ps{which}{h}_{n}")
                ps = kq_ps[key]
                for e in (3 * part, 3 * part + 1, 3 * part + 2):
                    nc.tensor.matmul(ps,
                                     lhsT=w_eh(e, h),
                                     rhs=xtb[n][:, e * 512:(e + 1) * 512],
                                     start=(e == 0), stop=(e == NE - 1))
                if part == 1:
                    nsl = slice(n * 512, (n + 1) * 512)
                    nc.vector.tensor_scalar_add(dst[h][:, nsl],
                                                kq_ps.pop(key),
                                                b_sb[:, h:h + 1])

            def emit_kq_chunk(h, n, which):
                for part in range(2):
                    emit_kq_part(h, n, which, part)

            def kq_chunks(h):
                for n in range(4):
                    yield ("k", h, n)
                    yield ("q", h, n)

            # ---- output projection chunk (one 128-row q tile) ----
            # Split across two 1-bank psums so it can borrow ps_proj slots;
            # PSUM->SBUF copies go on DVE (ACT is busy with exp here).
            # Tail out-proj chunks, two-phase: head 3's rows live only in
            # packed attnT tile 2, so the t0/t1 matmuls are independent of
            # the final normalization and run DURING its chain; only the t2
            # matmul (accumulation close) waits for the per-qm norm slice.
            # Psums borrow the attention rings (free by then).
            tail_ps = {}

            def emit_out_partial(qm, windowed=False):
                qsl = slice(qm * 128, (qm + 1) * 128)
                if windowed:
                    psA = ps_proj.tile([128, 512], f32_, tag="ps",
                                       name=f"poA{qm}")
                    psB = ps_proj.tile([128, 512], f32_, tag="ps",
                                       name=f"poB{qm}")
                else:
                    # tail only: both halves on the pss ring (free then; the
                    # pso ring is carrying the repack transposes)
                    psA = ps_pair.tile([128, 512], f32_, tag="pss",
                                       name=f"poA{qm}")
                    psB = ps_pair.tile([128, 512], f32_, tag="pss",
                                       name=f"poB{qm}")
                tail_ps[qm] = (psA, psB)
                for t in range(2):
                    nc.tensor.matmul(psA,
                                     lhsT=attnT[t][:, qsl],
                                     rhs=wo_t(t)[:, 0:512],
                                     start=(t == 0), stop=False)
                    nc.tensor.matmul(psB[:, 0:256],
                                     lhsT=attnT[t][:, qsl],
                                     rhs=wo_t(t)[:, 512:768],
                                     start=(t == 0), stop=False)

            def emit_out_finish(qm, windowed=False):
                qsl = slice(qm * 128, (qm + 1) * 128)
                psA, psB = tail_ps.pop(qm)
                nc.tensor.matmul(psA, lhsT=attnT[2][:, qsl],
                                 rhs=wo_t(2)[:, 0:512],
                                 start=False, stop=True)
                nc.tensor.matmul(psB[:, 0:256], lhsT=attnT[2][:, qsl],
                                 rhs=wo_t(2)[:, 512:768],
                                 start=False, stop=True)
                out_sb = outsb.tile([128, EMB], f32_, tag="osb",
                                    name=f"osb{qm}")
                # psA eviction on DVE (GPSIMD cannot touch PSUM on real
                # hw), psB on ACT (fits in its exp-stream slack)
                nc.vector.tensor_copy(out_sb[:, 0:512], psA)
                nc.sync.dma_start(out=outp[qm * 128:(qm + 1) * 128, 0:512],
                                  in_=out_sb[:, 0:512])
                nc.scalar.activation(out_sb[:, 512:768], psB[:, 0:256],
                                     mybir.ActivationFunctionType.Copy)
                nc.sync.dma_start(out=outp[qm * 128:(qm + 1) * 128, 512:768],
                                  in_=out_sb[:, 512:768])

            # ---- attention emit (with interleaved PE filler work) ----
            def emit_repack_qt(qc, qt):
                """Transpose normed[qc][qt] ([128 q, 384 packed head dims])
                into attnT via three 128x128 PE transposes + DVE evictions.
                Shares the ps ring with the out-proj partials; the interleave
                order keeps each allocation one eviction behind its slot's
                previous user."""
                tps = ps_proj.tile([128, 3 * 128], bf, tag="ps",
                                   name=f"tps{qc}_{qt}")
                for t_ in range(3):
                    nc.tensor.transpose(
                        tps[:, t_ * 128:(t_ + 1) * 128],
                        normed[qc][qt][:, t_ * 128:(t_ + 1) * 128],
                        ident_sb)
                csl = slice(qc * QC + qt * 128, qc * QC + (qt + 1) * 128)
                for t_ in range(3):
                    nc.vector.tensor_copy(attnT[t_][:, csl],
                                          tps[:, t_ * 128:(t_ + 1) * 128])

            def emit_attention(h, thunks_for_qc):
                """thunks_for_qc(qc) -> list of emit callables injected into
                the PE stream spread across this q-chunk's pairs."""
                for qc in range(NQC):
                    thunks = list(thunks_for_qc(qc))
                    inject_at = {}
                    if h == HPC - 1 and qc > 0:
                        # repack of the previous q-chunk interleaves with
                        # its P/F pairs; thunk order [rp0 rp1 P0 F0 rp2 rp3
                        # P1 F1 P2 F2 P3 F3] keeps every ps-ring allocation
                        # exactly one eviction behind its slot's last user
                        pos = [0, 0, 2, 3, 4, 4, 5, 5, 6, 6, 7, 7]
                        for i, t in enumerate(thunks):
                            inject_at.setdefault(
                                pos[i] if i < len(pos) else NPAIR - 1,
                                []).append(t)
                    else:
                        step = max(NPAIR // max(len(thunks), 1), 1)
                        for i, t in enumerate(thunks):
                            inject_at.setdefault(
                                min(1 + i * step, NPAIR - 1), []).append(t)
                    qsl = slice(qc * QC, (qc + 1) * QC)
                    idx = h * NQC + qc
                    # flipped-O accumulator: [128 q, 4 qt x (96 d + sums)]
                    # packed per q-tile into one psum bank.  The bank holds
                    # FOUR interleaved accumulation chains; hardware supports
                    # only one OPEN start/stop group per bank at a time
                    # (later start=True resets the others), so the tile is
                    # zeroed up front and every matmul accumulates with
                    # start=False.
                    psof = ps_o.tile([128, 4 * HDV], f32_, tag="pso",
                                     name=f"psof{idx}")
                    nc.vector.memset(psof, 0.0)
                    eps = []

                    def emit_ss(p):
                        pss = ps_pair.tile([128, 1024], f32_, tag="pss",
                                           name=f"pss{idx}_{p}")
                        ep = expp.tile([128, 1024], bf, tag="exp",
                                       name=f"exp{idx}_{p}")
                        # exp split: Pool takes the FIRST cols (waits only
                        # on the j=0 matmul, so its q7-launch latency hides
                        # behind the j=1 matmul), ACT takes the rest.  Both
                        # cadences stay under the pair's PE work so PE is
                        # the limiter.  Head 3 (qc>0) keeps exp fully on ACT
                        # - its windows carry out-proj thunks (PE/pair well
                        # above 1038 ns) and Pool absorbs psum evictions.
                        # both S matmuls BEFORE either exp: an exp emitted
                        # between them reads this pss tile and Tile's
                        # conservative subtile tracking then serializes the
                        # second matmul behind it (false intra-tile WAR)
                        nc.tensor.matmul(
                            pss[:, 0:512],
                            lhsT=kT[h][:, (2 * p) * 128:(2 * p + 1) * 128],
                            rhs=qT[h][:, qsl],
                            start=True, stop=True)
                        nc.tensor.matmul(
                            pss[:, 512:1024],
                            lhsT=kT[h][:, (2 * p + 1) * 128:
                                       (2 * p + 2) * 128],
                            rhs=qT[h][:, qsl],
                            start=True, stop=True)
                        nc.scalar.activation(
                            ep, pss,
                            mybir.ActivationFunctionType.Exp,
                            scale=SCALING)
                        eps.append(ep)

                    def emit_o(p):
                        # flipped: the exp tile is the (free) stationary
                        # load, the 97-col vaug slab streams -> 40 ns per
                        # matmul instead of 213, output lands [q, d]
                        for j in range(2):
                            kt = 2 * p + j
                            for qt in range(4):
                                nc.tensor.matmul(
                                    psof[:, qt * HDV:(qt + 1) * HDV],
                                    lhsT=eps[p][:, j * 512 + qt * 128:
                                                j * 512 + (qt + 1) * 128],
                                    rhs=vaug[kt][:, h * HDV:(h + 1) * HDV],
                                    start=False, stop=(kt == NKT - 1))

                    # O consumption trails S by TWO pairs: the Pool-half exp
                    # (sem + q7 launch + exec ~ 1 us latency) exceeds one
                    # pair's PE work; a 2-pair lag hides it completely
                    for p in range(NPAIR):
                        emit_ss(p)
                        for t in inject_at.get(p, ()):
                            t()
                        if p >= 3:
                            emit_o(p - 3)
                    emit_o(NPAIR - 3)
                    emit_o(NPAIR - 2)
                    emit_o(NPAIR - 1)

                    # normalization: sums live at col 96 of each q-tile
                    # group; reciprocal is a per-partition scalar (no
                    # broadcast machinery at all)
                    rbq = rbp.tile([128, 4], f32_, tag="rbq",
                                   name=f"rbq{idx}")
                    # one contiguous single-column reciprocal per q-tile
                    # (the custom-DVE op mis-lowers strided multi-column APs)
                    for qt in range(4):
                        nc.vector.reciprocal_approx_fast(
                            out=rbq[:, qt:qt + 1],
                            in_=psof[:, qt * HDV + HD:qt * HDV + HD + 1])
                    for qt in range(4):
                        nc.vector.tensor_scalar_mul(
                            normed[qc][qt][:, h * HD:(h + 1) * HD],
                            psof[:, qt * HDV:qt * HDV + HD],
                            rbq[:, qt:qt + 1])

            # ---- emission schedule ----
            # consume each x seq-block as its DMA lands: head 0's k/q chunks
            # for block n, then the V chunks of block n
            for n in range(4):
                emit_kq_chunk(0, n, "k")
                emit_kq_chunk(0, n, "q")
                for kt in range(4 * n, 4 * n + 4):
                    emit_v_chunk(kt)

            def kq_thunks(hnext):
                def f(qc):
                    # 2 chunks per q-chunk, split into 3-matmul halves (4
                    # thunks of ~640 ns) so the PE filler spreads across the
                    # window instead of lumping at two pairs
                    items = list(kq_chunks(hnext))[2 * qc:2 * qc + 2]
                    out = []
                    for it in items:
                        for part in range(2):
                            out.append(
                                lambda it=it, part=part: emit_kq_part(
                                    it[1], it[2], it[0], part))
                    return out
                return f

            def out_thunks(qc):
                # during h3's q-chunk qc: repack q-chunk qc-1 from normed
                # into attnT, then run its out-proj chunks two-phase (P =
                # t0/t1 partials, F = t2 close + evictions + stores)
                if qc == 0:
                    return []
                qp = qc - 1
                out = []
                for qt4 in range(0, 4, 2):
                    out.append(lambda qt=qt4: emit_repack_qt(qp, qt))
                    out.append(lambda qt=qt4 + 1: emit_repack_qt(qp, qt))
                    for qm in (4 * qp + qt4, 4 * qp + qt4 + 1):
                        out.append(lambda qm=qm: emit_out_partial(qm, True))
                        out.append(lambda qm=qm: emit_out_finish(qm, True))
                return out

            for h in range(HPC - 1):
                emit_attention(h, kq_thunks(h + 1))
            emit_attention(HPC - 1, out_thunks)
            # tail: repack + out-proj of the final q-chunk; partials
            # alternate between the ps and pss rings so two stay in flight
            q0 = 4 * (NQC - 1)
            emit_repack_qt(NQC - 1, 0)
            emit_repack_qt(NQC - 1, 1)
            emit_out_partial(q0)
            emit_repack_qt(NQC - 1, 2)
            emit_repack_qt(NQC - 1, 3)
            emit_out_finish(q0)
            emit_out_partial(q0 + 1, True)
            emit_out_finish(q0 + 1)
            emit_out_partial(q0 + 2)
            emit_out_finish(q0 + 2)
            emit_out_partial(q0 + 3, True)
            emit_out_finish(q0 + 3)

            if os.environ.get("KDEBUG"):
                dbg_attn = nc.dram_tensor(
                    "dbg_attn", [3 * 128, SEQ], bf,
                    kind="ExternalOutput").ap()
                for t_ in range(3):
                    nc.sync.dma_start(
                        out=dbg_attn[t_ * 128:(t_ + 1) * 128, :],
                        in_=attnT[t_])
                dbg_nm = nc.dram_tensor(
                    "dbg_nm", [NQC * 4 * 128, HPC * HD], bf,
                    kind="ExternalOutput").ap()
                for qc in range(NQC):
                    for qt in range(4):
                        r0 = (qc * 4 + qt) * 128
                        nc.sync.dma_start(out=dbg_nm[r0:r0 + 128, :],
                                          in_=normed[qc][qt])

    nc.compile()
    return nc


def _get_nc():
    if "nc" not in _NC_CACHE:
        _NC_CACHE["nc"] = _build_nc()
    return _NC_CACHE["nc"]


def _etile_pack(wT):
    """[768, n] (e on rows) -> [128, 6*n] bf16: e-tiles as column blocks so
    the whole operand loads as ONE [128, n] DMA."""
    n = wT.shape[1]
    a = wT.reshape(NE, 128, n).transpose(1, 0, 2)
    return np.ascontiguousarray(a.reshape(128, NE * n)).astype(BF16)


def _x_block_pack(x_b):
    """[2048, 768] x -> [128, 4 * 6 * 512] bf16, seq-block-major: block n
    holds e-tiles of sequence rows n*512..(n+1)*512 as column slabs."""
    a = x_b.reshape(4, 512, NE, 128)          # n, s, e, p
    a = a.transpose(3, 0, 2, 1)               # p, n, e, s
    return np.ascontiguousarray(a.reshape(128, 4 * NE * 512)).astype(BF16)


def _pad_headsT(w_rows):
    """[384, 768] head rows -> zero-pad head dim 96->128 -> transpose -> [768, 512]."""
    p = np.zeros((HPC * HDP, EMB), np.float32)
    p.reshape(HPC, HDP, EMB)[:, :HD] = w_rows.reshape(HPC, HD, EMB)
    return np.ascontiguousarray(p.T)


def _pad_bias(b_rows):
    """[384] head bias -> [128, HPC] padded/transposed for per-partition add."""
    p = np.zeros((HPC, HDP), np.float32)
    p[:, :HD] = b_rows.reshape(HPC, HD)
    return np.ascontiguousarray(p.T)


def kernel(x, Wq, bq, Wk, bk, Wv, bv, Wo, bo):
    x = np.asarray(x, np.float32)
    Wq, bq = np.asarray(Wq, np.float32), np.asarray(bq, np.float32)
    Wk, bk = np.asarray(Wk, np.float32), np.asarray(bk, np.float32)
    Wv, bv = np.asarray(Wv, np.float32), np.asarray(bv, np.float32)
    Wo, bo = np.asarray(Wo, np.float32), np.asarray(bo, np.float32)

    nc = _get_nc()

    in_maps = []
    for c in range(NCORES):
        b, hg = divmod(c, 2)
        hs = slice(hg * HPC * HD, (hg + 1) * HPC * HD)
        woT = Wo[:, hs].T  # [384, 768]
        wo_pack = np.ascontiguousarray(
            woT.reshape(3, 128, EMB).transpose(1, 0, 2).reshape(128, 3 * EMB))
        wq_et = _etile_pack(_pad_headsT(Wq[hs])).reshape(128, NE, HPC, HDP)
        wk_et = _etile_pack(_pad_headsT(Wk[hs])).reshape(128, NE, HPC, HDP)
        in_maps.append({
            "xtp": _x_block_pack(x[b]),
            "wqp0": np.ascontiguousarray(
                wq_et[:, :, 0].reshape(128, NE * HDP)),
            "wqpr": np.ascontiguousarray(
                wq_et[:, :, 1:].reshape(128, NE * 3 * HDP)),
            "wkp0": np.ascontiguousarray(
                wk_et[:, :, 0].reshape(128, NE * HDP)),
            "wkpr": np.ascontiguousarray(
                wk_et[:, :, 1:].reshape(128, NE * 3 * HDP)),
            "wvp": _etile_pack(np.ascontiguousarray(Wv[hs].T)),
            "wop": wo_pack.astype(BF16),
            "bqp": _pad_bias(bq[hs]),
            "bkp": _pad_bias(bk[hs]),
            "identp": np.ascontiguousarray(np.eye(128, dtype=np.float32))
            .astype(BF16),
        })

    global LAST_RESULT
    trace = bool(int(os.environ.get("KERNEL_TRACE", "0")))
    tmpdir = os.environ.get("KERNEL_TRACE_DIR") or None
    res = run_bass_kernel_spmd(nc, in_maps, list(range(NCORES)), trace=trace,
                               tmpdir=tmpdir)
    LAST_RESULT = res

    out = np.empty((B, SEQ, EMB), np.float32)
    for b in range(B):
        out[b] = res.results[2 * b]["outp"] + res.results[2 * b + 1]["outp"]
    # bv enters each head's output additively (sum of softmax weights is 1),
    # and bo is a plain add: both fold into one constant vector.
    out += Wo @ bv + bo
    return out



# revision 77
# speedup vs baseline: 1.0740x; 1.0053x over previous
"""Multi-head attention (B=4, S=2048, E=768, H=8, D=96) on 8 Trainium2 cores.

Sharding: core c -> (batch b = c//2, head-group hg = c%2 of 4 heads).
Each core computes Q/K/V projections for its 4 heads over the full sequence
of its batch, full attention for those heads, and a partial output
projection (row-split Wo).  The two cores of a batch produce partial
outputs that are summed on the host during unsharding (tensor-parallel
reduce).

On-chip layout notes:
  - All matmul operands are bf16 (1 cycle/row on PE; fp32 would be 4x;
    fp8 DoubleRow would halve PE time but its ~3% per-element quantization
    noise lands ~1:1 in the output - the softmax average does not shrink
    relative error for zero-mean V - and busts the 2e-2 tolerance).
  - head_dim 96 is zero-padded to 128 (host pads Wq/Wk columns), so every
    matmul has K=128 contraction and 128-column stationary operands (FWL).
  - Attention scores are computed transposed, S^T[k, q] = K^T.T @ Q^T,
    so softmax normalization is a partition reduction; we get the sums for
    free by augmenting V with a ones column (row 96 of the O^T accumulator
    is then sum_k exp(S)).
  - exp runs on the scalar engine straight out of PSUM ([128,1024] over a
    pair of key tiles) with the 1/sqrt(d) scale folded into the
    activation's scale parameter.  PE is the bottleneck engine (~88%
    busy); ACT has slack, so it also absorbs some PSUM evictions.
  - Per-(head, q-chunk) normalization: sums row -> DRAM -> broadcast-DMA
    to 96 partitions -> fast reciprocal on DVE -> tensor_tensor mults.
    The two sums DMAs ride the Pool/SWDGE queue so they never head-of-line
    block the SP queue that carries input loads and output stores.  For
    the LAST head the roundtrip (~6 us) would stall the dependent
    output-projection chunks, so there the sums row is broadcast across
    96 partitions with a K=1 PE matmul against a ones column (~0.3 us),
    and the normalization multiplies are sliced per 128-column
    output-projection tile so each tile unblocks as early as possible.
  - Inputs are host-packed so each operand is ONE [128, n] DMA (the SP
    sequencer spends ~650 ns per DMA issue; 47 small input DMAs would
    serialize into a ~26 us head).  x is packed seq-block-major so the
    first K/Q chunk needs only wk + 0.75MB; biases load early because
    they gate the K/Q psum evictions; head 0's K/Q weight columns load
    before the other heads'.
  - PE stream order: head 0's K/Q projections first (they gate the ACT
    exp stream), V projection chunks interleaved per x block, then the
    attention pipeline; K/Q chunks of head h+1 are interleaved into head
    h's attention stream.  Output-projection chunks can only run
    during/after the last head (they need all four heads' softmax
    normalizations), but head 3's attnT rows live solely in packed tile
    2 - so each chunk splits into a t0/t1 partial (independent of the
    newest normalization, injected at the bare front pairs of the
    following q-chunk's stream) and a t2 finish (accumulation close +
    evictions + split stores) injected from pair 3.  The final q-chunk's
    four chunks run the same two-phase way in the tail, borrowing the
    then-idle attention psum rings so the partials overlap the last
    normalization chain.
"""

import os
import sys

sys.path.insert(0, "/opt/trn_rl_repo")

import numpy as np
import ml_dtypes

import concourse.bacc as bacc
import concourse.bass as bass
import concourse.tile as tile
from concourse import mybir
from concourse.bass_utils import run_bass_kernel_spmd

BF16 = ml_dtypes.bfloat16


def _pool_activation(nc, out, in_, func, scale=1.0):
    """Emit an InstActivation on the Pool/GPSIMD engine (bass only defines
    activation() on the scalar engine; the instruction itself is engine-
    tagged by the emitting queue, and the gpsimd DSP implements it)."""
    return bass.BassScalarEngine.activation(
        nc.gpsimd, out, in_, func, scale=scale)

EMB = 768
HEADS = 8
HD = 96          # true head dim
HDP = 128        # padded head dim
SEQ = 2048
B = 4
NCORES = 8
HPC = 4          # heads per core
SCALING = HD ** -0.5
QC = 512         # query chunk per attention inner loop
NQC = SEQ // QC
NKT = SEQ // 128  # 16 key tiles
NPAIR = NKT // 2
NE = EMB // 128   # 6 e_in tiles

_NC_CACHE = {}
LAST_RESULT = None  # BassKernelResults of the most recent run (for test.py)


def _build_nc():
    f32 = mybir.dt.float32
    bf = mybir.dt.bfloat16

    nc = bacc.Bacc(trn_type="TRN2", target_bir_lowering=False, debug=False,
                   num_devices=NCORES)

    # All operands host-packed into [128, n] so each loads as ONE DMA.
    xtp = nc.dram_tensor("xtp", [128, NE * SEQ], bf, kind="ExternalInput").ap()
    # K/Q weights split: head 0's columns load first (0.19MB) so head 0's
    # projections - which gate the whole pipeline - start ~5 us earlier
    wqp0 = nc.dram_tensor("wqp0", [128, NE * HDP], bf,
                          kind="ExternalInput").ap()
    wqpr = nc.dram_tensor("wqpr", [128, NE * 3 * HDP], bf,
                          kind="ExternalInput").ap()
    wkp0 = nc.dram_tensor("wkp0", [128, NE * HDP], bf,
                          kind="ExternalInput").ap()
    wkpr = nc.dram_tensor("wkpr", [128, NE * 3 * HDP], bf,
                          kind="ExternalInput").ap()
    wvp = nc.dram_tensor("wvp", [128, NE * HPC * HD], bf,
                         kind="ExternalInput").ap()
    wop = nc.dram_tensor("wop", [128, 3 * EMB], bf, kind="ExternalInput").ap()
    bqp = nc.dram_tensor("bqp", [128, HPC], f32, kind="ExternalInput").ap()
    bkp = nc.dram_tensor("bkp", [128, HPC], f32, kind="ExternalInput").ap()
    identp = nc.dram_tensor("identp", [128, 128], bf,
                            kind="ExternalInput").ap()
    outp = nc.dram_tensor("outp", [SEQ, EMB], f32, kind="ExternalOutput").ap()

    with tile.TileContext(nc) as tc:
        with (
            tc.tile_pool(name="const", bufs=1) as constp,
            tc.tile_pool(name="big", bufs=1) as bigp,
            tc.tile_pool(name="expp", bufs=6) as expp,
            tc.tile_pool(name="rbp", bufs=4) as rbp,
            tc.tile_pool(name="normp", bufs=1) as normp,
            tc.tile_pool(name="outsb", bufs=12) as outsb,
            tc.tile_pool(name="ps_proj", bufs=2, space="PSUM") as ps_proj,
            tc.tile_pool(name="ps_o", bufs=2, space="PSUM") as ps_o,
            tc.tile_pool(name="ps_pair", bufs=2, space="PSUM") as ps_pair,
        ):
            # ---- loads. x is packed seq-block-major ([128, 6e x 512] per
            # 512-sequence block) so the first K/Q chunk only needs wk + one
            # 0.75MB block; wk + block 0/1 load first, k/q chunks of head 0
            # then pipeline behind the remaining block DMAs. ----
            XB = NE * 512  # 3072 cols per seq block
            wk0_sb = constp.tile([128, NE * HDP], bf, name="wk0_sb")
            nc.sync.dma_start(out=wk0_sb, in_=wkp0)
            xtb = [bigp.tile([128, XB], bf, name=f"xtb{n}") for n in range(4)]
            nc.sync.dma_start(out=xtb[0][:, 0:XB // 2], in_=xtp[:, 0:XB // 2])
            nc.sync.dma_start(out=xtb[0][:, XB // 2:XB],
                              in_=xtp[:, XB // 2:XB])
            # biases next: tiny, but they gate the K/Q psum evictions
            bq_sb = constp.tile([128, HPC], f32, name="bq_sb")
            nc.sync.dma_start(out=bq_sb, in_=bqp)
            bk_sb = constp.tile([128, HPC], f32, name="bk_sb")
            nc.sync.dma_start(out=bk_sb, in_=bkp)
            wq0_sb = constp.tile([128, NE * HDP], bf, name="wq0_sb")
            nc.sync.dma_start(out=wq0_sb, in_=wqp0)
            wv_sb = constp.tile([128, NE * HPC * HD], bf, name="wv_sb")
            nc.sync.dma_start(out=wv_sb, in_=wvp)
            nc.sync.dma_start(out=xtb[1], in_=xtp[:, XB:2 * XB])
            wkr_sb = constp.tile([128, NE * 3 * HDP], bf, name="wkr_sb")
            nc.sync.dma_start(out=wkr_sb, in_=wkpr)
            wqr_sb = constp.tile([128, NE * 3 * HDP], bf, name="wqr_sb")
            nc.sync.dma_start(out=wqr_sb, in_=wqpr)
            nc.sync.dma_start(out=xtb[2], in_=xtp[:, 2 * XB:3 * XB])
            nc.sync.dma_start(out=xtb[3], in_=xtp[:, 3 * XB:4 * XB])
            wo_sb = constp.tile([128, 3 * EMB], bf, name="wo_sb")
            nc.sync.dma_start(out=wo_sb, in_=wop)
            # identity for the PE repack transposes (first needed in head
            # 3's phase, so it loads last)
            ident_sb = constp.tile([128, 128], bf, name="ident_sb")
            nc.sync.dma_start(out=ident_sb, in_=identp)

            def wk_eh(e, h):
                if h == 0:
                    return wk0_sb[:, e * HDP:(e + 1) * HDP]
                return wkr_sb[:, (e * 3 + h - 1) * HDP:(e * 3 + h) * HDP]

            def wq_eh(e, h):
                if h == 0:
                    return wq0_sb[:, e * HDP:(e + 1) * HDP]
                return wqr_sb[:, (e * 3 + h - 1) * HDP:(e * 3 + h) * HDP]

            def wv_e(e):
                return wv_sb[:, e * HPC * HD:(e + 1) * HPC * HD]

            def wo_t(t_):
                return wo_sb[:, t_ * EMB:(t_ + 1) * EMB]

            # ---- persistent intermediates ----
            # vaug: per key tile, 4 heads x (96 v-cols + a ones col).  The
            # ones col makes row^T @ vaug yield the softmax denominator in
            # the same accumulator (col 96 of each head group).  No pad
            # cols: vaug is the MOVING operand of the flipped O matmul, so
            # narrower means cheaper (97 vs 128 cycles).
            HDV = HD + 1
            vaug = []
            for kt in range(NKT):
                t = bigp.tile([128, HPC * HDV], bf, name=f"vaug{kt}")
                ones_cols = t.rearrange("p (h c) -> p h c",
                                        h=HPC)[:, :, HD:HD + 1]
                nc.gpsimd.memset(ones_cols, 1.0)
                vaug.append(t)
            qT = [bigp.tile([128, SEQ], bf, name=f"qT{h}") for h in range(HPC)]
            kT = [bigp.tile([128, SEQ], bf, name=f"kT{h}") for h in range(HPC)]
            # packed attention output, [384 rows = 3 tiles x 128, seq],
            # written by the h3-phase repack transposes
            attnT = [bigp.tile([128, SEQ], bf, name=f"attnT{t_}")
                     for t_ in range(3)]
            # normalized O in [q, head-dims] layout, per (q-chunk, q-tile):
            # written per head as its window completes, transposed into
            # attnT once all four heads are in
            normed = [[normp.tile([128, HPC * HD], bf, name=f"nm{qc}_{qt}")
                       for qt in range(4)] for qc in range(NQC)]

            f32_ = f32

            # ---- projection emit helpers ----
            def emit_v_chunk(kt):
                psv = ps_proj.tile([128, 512], f32_, tag="ps",
                                   name=f"psv{kt}")
                blk, off = divmod(kt, 4)
                for e in range(NE):
                    nc.tensor.matmul(psv[:, 0:HPC * HD],
                                     lhsT=xtb[blk][:, e * 512 + off * 128:
                                                   e * 512 + off * 128 + 128],
                                     rhs=wv_e(e),
                                     start=(e == 0), stop=(e == NE - 1))
                for hh in range(HPC):
                    nc.vector.tensor_copy(
                        vaug[kt][:, hh * HDV:hh * HDV + HD],
                        psv[:, hh * HD:(hh + 1) * HD])

            kq_ps = {}

            def emit_kq_part(h, n, which, part):
                """One half of a K/Q chunk (3 of 6 e-matmuls).  Thunking
                chunks at this granularity keeps the per-pair PE load above
                the ACT exp cadence, so the S stream never outruns ACT into
                the 2-slot pss ring."""
                key = (h, n, which)
                w_eh, dst, b_sb = ((wk_eh, kT, bk_sb) if which == "k"
                                   else (wq_eh, qT, bq_sb))
                if part == 0:
                    kq_ps[key] = ps_proj.tile([128, 512], f32_, tag="ps",
                                              name=f"ps{which}{h}_{n}")
                ps = kq_ps[key]
                for e in (3 * part, 3 * part + 1, 3 * part + 2):
                    nc.tensor.matmul(ps,
                                     lhsT=w_eh(e, h),
                                     rhs=xtb[n][:, e * 512:(e + 1) * 512],
                                     start=(e == 0), stop=(e == NE - 1))
                if part == 1:
                    nsl = slice(n * 512, (n + 1) * 512)
                    nc.vector.tensor_scalar_add(dst[h][:, nsl],
                                                kq_ps.pop(key),
                                                b_sb[:, h:h + 1])

            def emit_kq_chunk(h, n, which):
                for part in range(2):
                    emit_kq_part(h, n, which, part)

            def kq_chunks(h):
                for n in range(4):
                    yield ("k", h, n)
                    yield ("q", h, n)

            # ---- output projection chunk (one 128-row q tile) ----
            # Split across two 1-bank psums so it can borrow ps_proj slots;
            # PSUM->SBUF copies go on DVE (ACT is busy with exp here).
            # Tail out-proj chunks, two-phase: head 3's rows live only in
            # packed attnT tile 2, so the t0/t1 matmuls are independent of
            # the final normalization and run DURING its chain; only the t2
            # matmul (accumulation close) waits for the per-qm norm slice.
            # Psums borrow the attention rings (free by then).
            tail_ps = {}

            def emit_out_partial(qm, mode="tail"):
                qsl = slice(qm * 128, (qm + 1) * 128)
                if mode == "win":
                    psA = ps_proj.tile([128, 512], f32_, tag="ps",
                                       name=f"poA{qm}")
                    psB = ps_proj.tile([128, 512], f32_, tag="ps",
                                       name=f"poB{qm}")
                elif mode == "edge":
                    # last-window partial: psA on the ps ring, psB on the
                    # pso ring's spare slot (the previous psof freed once
                    # its normalization drained) so only ONE ps slot is
                    # held across the window/tail boundary
                    psA = ps_proj.tile([128, 512], f32_, tag="ps",
                                       name=f"poA{qm}")
                    psB = ps_o.tile([128, 512], f32_, tag="pso",
                                    name=f"poB{qm}")
                else:
                    # tail: both halves on the pss ring (free then)
                    psA = ps_pair.tile([128, 512], f32_, tag="pss",
                                       name=f"poA{qm}")
                    psB = ps_pair.tile([128, 512], f32_, tag="pss",
                                       name=f"poB{qm}")
                tail_ps[qm] = (psA, psB)
                for t in range(2):
                    nc.tensor.matmul(psA,
                                     lhsT=attnT[t][:, qsl],
                                     rhs=wo_t(t)[:, 0:512],
                                     start=(t == 0), stop=False)
                    nc.tensor.matmul(psB[:, 0:256],
                                     lhsT=attnT[t][:, qsl],
                                     rhs=wo_t(t)[:, 512:768],
                                     start=(t == 0), stop=False)

            def emit_out_finish(qm, windowed=False):
                qsl = slice(qm * 128, (qm + 1) * 128)
                psA, psB = tail_ps.pop(qm)
                nc.tensor.matmul(psA, lhsT=attnT[2][:, qsl],
                                 rhs=wo_t(2)[:, 0:512],
                                 start=False, stop=True)
                nc.tensor.matmul(psB[:, 0:256], lhsT=attnT[2][:, qsl],
                                 rhs=wo_t(2)[:, 512:768],
                                 start=False, stop=True)
                out_sb = outsb.tile([128, EMB], f32_, tag="osb",
                                    name=f"osb{qm}")
                # evictions on DVE in windows (ACT is exp-saturated there;
                # GPSIMD cannot touch PSUM on real hw).  In the tail, psB
                # rides ACT end-to-end (evict + store) so the last stores
                # split across the SP and ACT queues instead of
                # serializing on SP's 650 ns issue cost.
                nc.vector.tensor_copy(out_sb[:, 0:512], psA)
                if windowed:
                    nc.sync.dma_start(
                        out=outp[qm * 128:(qm + 1) * 128, 0:512],
                        in_=out_sb[:, 0:512])
                else:
                    eng = nc.gpsimd if qm % 2 == 0 else nc.sync
                    eng.dma_start(
                        out=outp[qm * 128:(qm + 1) * 128, 0:512],
                        in_=out_sb[:, 0:512])
                if windowed:
                    nc.vector.tensor_copy(out_sb[:, 512:768], psB[:, 0:256])
                    nc.sync.dma_start(
                        out=outp[qm * 128:(qm + 1) * 128, 512:768],
                        in_=out_sb[:, 512:768])
                else:
                    # tail: psB evict on DVE; store rotated across the
                    # ACT and Pool/SWDGE queues so the four final stores
                    # don't serialize on the single HWDGE slot
                    nc.vector.tensor_copy(out_sb[:, 512:768], psB[:, 0:256])
                    eng = nc.scalar if qm % 2 == 0 else nc.gpsimd
                    eng.dma_start(
                        out=outp[qm * 128:(qm + 1) * 128, 512:768],
                        in_=out_sb[:, 512:768])

            # ---- attention emit (with interleaved PE filler work) ----
            # repack: transpose normed[qc][qt] ([128 q, 384 packed head
            # dims]) into attnT via 128x128 PE transposes + DVE evictions.
            # Chunks t0/t1 cover only heads 0-2, so they repack as soon as
            # head 2's normalization lands - during head 3's own windows -
            # and the out-proj partials (which read t0/t1) can then run
            # inside the last window.  Only the t2 chunk (heads 2+3) waits
            # for head 3's normalization.
            def emit_repack01(qc, qt):
                tps = ps_proj.tile([128, 2 * 128], bf, tag="ps",
                                   name=f"tp01_{qc}_{qt}")
                for t_ in range(2):
                    nc.tensor.transpose(
                        tps[:, t_ * 128:(t_ + 1) * 128],
                        normed[qc][qt][:, t_ * 128:(t_ + 1) * 128],
                        ident_sb)
                csl = slice(qc * QC + qt * 128, qc * QC + (qt + 1) * 128)
                for t_ in range(2):
                    nc.vector.tensor_copy(attnT[t_][:, csl],
                                          tps[:, t_ * 128:(t_ + 1) * 128])

            def emit_repack2(qc, qt, tail=False):
                tps = ps_proj.tile([128, 128], bf, tag="ps",
                                   name=f"tp2_{qc}_{qt}")
                nc.tensor.transpose(tps, normed[qc][qt][:, 256:384],
                                    ident_sb)
                csl = slice(qc * QC + qt * 128, qc * QC + (qt + 1) * 128)
                if tail:
                    # keep the tail's DVE queue clear for the out evictions
                    nc.scalar.activation(attnT[2][:, csl], tps,
                                         mybir.ActivationFunctionType.Copy)
                else:
                    nc.vector.tensor_copy(attnT[2][:, csl], tps)

            # cross-window O backlog: each pair's O-block (and, for a
            # window's last pair, its normalization) is queued and drained
            # `lag` pairs later - ACROSS window boundaries for heads 0-2,
            # so a window's trailing O's overlap the next window's S
            # stream.  Head-3 windows drain fully at their end because the
            # following window's repack thunks need the normalization.
            obl = []

            def emit_attention(h, thunks_for_qc):
                """thunks_for_qc(qc) -> list of emit callables injected into
                the PE stream spread across this q-chunk's pairs."""
                for qc in range(NQC):
                    thunks, pos = thunks_for_qc(qc)
                    inject_at = {}
                    if pos is not None:
                        for i, t in enumerate(thunks):
                            inject_at.setdefault(
                                pos[i] if i < len(pos) else NPAIR - 1,
                                []).append(t)
                    else:
                        step = max(NPAIR // max(len(thunks), 1), 1)
                        for i, t in enumerate(thunks):
                            inject_at.setdefault(
                                min(1 + i * step, NPAIR - 1), []).append(t)
                    qsl = slice(qc * QC, (qc + 1) * QC)
                    idx = h * NQC + qc
                    # flipped-O accumulator: [128 q, 4 qt x (96 d + sums)]
                    # packed per q-tile into one psum bank.  The bank holds
                    # FOUR interleaved accumulation chains; hardware supports
                    # only one OPEN start/stop group per bank at a time
                    # (later start=True resets the others), so the tile is
                    # zeroed up front and every matmul accumulates with
                    # start=False.
                    psof = ps_o.tile([128, 4 * HDV], f32_, tag="pso",
                                     name=f"psof{idx}")
                    nc.vector.memset(psof, 0.0)
                    eps = []

                    def emit_ss(p):
                        pss = ps_pair.tile([128, 1024], f32_, tag="pss",
                                           name=f"pss{idx}_{p}")
                        ep = expp.tile([128, 1024], bf, tag="exp",
                                       name=f"exp{idx}_{p}")
                        # exp split: Pool takes the FIRST cols (waits only
                        # on the j=0 matmul, so its q7-launch latency hides
                        # behind the j=1 matmul), ACT takes the rest.  Both
                        # cadences stay under the pair's PE work so PE is
                        # the limiter.  Head 3 (qc>0) keeps exp fully on ACT
                        # - its windows carry out-proj thunks (PE/pair well
                        # above 1038 ns) and Pool absorbs psum evictions.
                        # both S matmuls BEFORE either exp: an exp emitted
                        # between them reads this pss tile and Tile's
                        # conservative subtile tracking then serializes the
                        # second matmul behind it (false intra-tile WAR)
                        nc.tensor.matmul(
                            pss[:, 0:512],
                            lhsT=kT[h][:, (2 * p) * 128:(2 * p + 1) * 128],
                            rhs=qT[h][:, qsl],
                            start=True, stop=True)
                        nc.tensor.matmul(
                            pss[:, 512:1024],
                            lhsT=kT[h][:, (2 * p + 1) * 128:
                                       (2 * p + 2) * 128],
                            rhs=qT[h][:, qsl],
                            start=True, stop=True)
                        nc.scalar.activation(
                            ep, pss,
                            mybir.ActivationFunctionType.Exp,
                            scale=SCALING)
                        eps.append(ep)

                    def emit_o(p, ep, psof=psof, h=h):
                        # flipped: the exp tile is the (free) stationary
                        # load, the 97-col vaug slab streams -> 40 ns per
                        # matmul instead of 213, output lands [q, d]
                        for j in range(2):
                            kt = 2 * p + j
                            for qt in range(4):
                                nc.tensor.matmul(
                                    psof[:, qt * HDV:(qt + 1) * HDV],
                                    lhsT=ep[:, j * 512 + qt * 128:
                                            j * 512 + (qt + 1) * 128],
                                    rhs=vaug[kt][:, h * HDV:(h + 1) * HDV],
                                    start=False, stop=(kt == NKT - 1))

                    def emit_norm(psof=psof, h=h, qc=qc, idx=idx):
                        # sums live at col 96 of each q-tile group; the
                        # reciprocal is a per-partition scalar.  One
                        # contiguous single-column reciprocal per q-tile
                        # (the custom-DVE op mis-lowers strided APs).
                        rbq = rbp.tile([128, 4], f32_, tag="rbq",
                                       name=f"rbq{idx}")
                        for qt in range(4):
                            nc.vector.reciprocal_approx_fast(
                                out=rbq[:, qt:qt + 1],
                                in_=psof[:, qt * HDV + HD:qt * HDV + HD + 1])
                        for qt in range(4):
                            nc.vector.tensor_scalar_mul(
                                normed[qc][qt][:, h * HD:(h + 1) * HD],
                                psof[:, qt * HDV:qt * HDV + HD],
                                rbq[:, qt:qt + 1])

                    def make_o(p, last, ep):
                        def f():
                            emit_o(p, ep)
                            if last:
                                emit_norm()
                        return f

                    lag = 4
                    for p in range(NPAIR):
                        emit_ss(p)
                        for t in inject_at.get(p, ()):
                            t()
                        obl.append(make_o(p, p == NPAIR - 1, eps[-1]))
                        while len(obl) > lag:
                            obl.pop(0)()
                    while obl:
                        obl.pop(0)()

            # ---- emission schedule ----
            # consume each x seq-block as its DMA lands: head 0's k/q chunks
            # for block n, then the V chunks of block n
            for n in range(4):
                emit_kq_chunk(0, n, "k")
                emit_kq_chunk(0, n, "q")
                for kt in range(4 * n, 4 * n + 4):
                    emit_v_chunk(kt)

            def kq_thunks(hnext):
                def f(qc):
                    # 2 chunks per q-chunk, split into 3-matmul halves (4
                    # thunks of ~640 ns) so the PE filler spreads across the
                    # window instead of lumping at two pairs
                    items = list(kq_chunks(hnext))[2 * qc:2 * qc + 2]
                    out = []
                    for it in items:
                        for part in range(2):
                            out.append(
                                lambda it=it, part=part: emit_kq_part(
                                    it[1], it[2], it[0], part))
                    return out, None
                return f

            def rp01_thunks(qc):
                return [lambda qt=qt: emit_repack01(qc, qt)
                        for qt in range(4)]

            def out_thunks(qc):
                # head-3 window qc: close out q-chunk qc-1 (t2 repack + P/F
                # two-phase out-proj) and pre-repack t0/t1 of q-chunk qc+1
                # (they only need heads 0-2, normalized long ago).  The
                # last window also starts P(q0+0) with its psB on the pso
                # spare slot so the tail begins with a partial in flight.
                if qc == 0:
                    return (rp01_thunks(0) + rp01_thunks(1),
                            [0, 1, 2, 3, 4, 5, 6, 7])
                qp = qc - 1
                out = []
                for qt4 in range(0, 4, 2):
                    out.append(lambda qt=qt4: emit_repack2(qp, qt))
                    out.append(lambda qt=qt4 + 1: emit_repack2(qp, qt))
                    for qm in (4 * qp + qt4, 4 * qp + qt4 + 1):
                        out.append(
                            lambda qm=qm: emit_out_partial(qm, "win"))
                        out.append(lambda qm=qm: emit_out_finish(qm, True))
                pos = [0, 0, 1, 2, 3, 3, 4, 4, 5, 5, 6, 6]
                if qc < NQC - 1:
                    out += rp01_thunks(qc + 1)
                    pos += [7, 7, 7, 7]
                else:
                    out.append(lambda: emit_out_partial(4 * qc, "edge"))
                    pos += [7]
                return out, pos

            for h in range(HPC - 1):
                emit_attention(h, kq_thunks(h + 1))
            emit_attention(HPC - 1, out_thunks)
            # tail: only the t2 repacks and the finish halves remain; each
            # F(qm) frees the ps/pso slots its successor partial needs
            q0 = 4 * (NQC - 1)
            emit_repack2(NQC - 1, 0, True)
            emit_out_finish(q0)
            emit_out_partial(q0 + 1)
            emit_repack2(NQC - 1, 1, True)
            emit_out_finish(q0 + 1)
            emit_out_partial(q0 + 2)
            emit_repack2(NQC - 1, 2, True)
            emit_out_finish(q0 + 2)
            emit_out_partial(q0 + 3)
            emit_repack2(NQC - 1, 3, True)
            emit_out_finish(q0 + 3)

            if os.environ.get("KDEBUG"):
                dbg_attn = nc.dram_tensor(
                    "dbg_attn", [3 * 128, SEQ], bf,
                    kind="ExternalOutput").ap()
                for t_ in range(3):
                    nc.sync.dma_start(
                        out=dbg_attn[t_ * 128:(t_ + 1) * 128, :],
                        in_=attnT[t_])
                dbg_nm = nc.dram_tensor(
                    "dbg_nm", [NQC * 4 * 128, HPC * HD], bf,
                    kind="ExternalOutput").ap()
                for qc in range(NQC):
                    for qt in range(4):
                        r0 = (qc * 4 + qt) * 128
                        nc.sync.dma_start(out=dbg_nm[r0:r0 + 128, :],
                                          in_=normed[qc][qt])

    nc.compile()
    return nc


def _get_nc():
    if "nc" not in _NC_CACHE:
        _NC_CACHE["nc"] = _build_nc()
    return _NC_CACHE["nc"]


def _etile_pack(wT):
    """[768, n] (e on rows) -> [128, 6*n] bf16: e-tiles as column blocks so
    the whole operand loads as ONE [128, n] DMA."""
    n = wT.shape[1]
    a = wT.reshape(NE, 128, n).transpose(1, 0, 2)
    return np.ascontiguousarray(a.reshape(128, NE * n)).astype(BF16)


def _x_block_pack(x_b):
    """[2048, 768] x -> [128, 4 * 6 * 512] bf16, seq-block-major: block n
    holds e-tiles of sequence rows n*512..(n+1)*512 as column slabs."""
    a = x_b.reshape(4, 512, NE, 128)          # n, s, e, p
    a = a.transpose(3, 0, 2, 1)               # p, n, e, s
    return np.ascontiguousarray(a.reshape(128, 4 * NE * 512)).astype(BF16)


def _pad_headsT(w_rows):
    """[384, 768] head rows -> zero-pad head dim 96->128 -> transpose -> [768, 512]."""
    p = np.zeros((HPC * HDP, EMB), np.float32)
    p.reshape(HPC, HDP, EMB)[:, :HD] = w_rows.reshape(HPC, HD, EMB)
    return np.ascontiguousarray(p.T)


def _pad_bias(b_rows):
    """[384] head bias -> [128, HPC] padded/transposed for per-partition add."""
    p = np.zeros((HPC, HDP), np.float32)
    p[:, :HD] = b_rows.reshape(HPC, HD)
    return np.ascontiguousarray(p.T)


def kernel(x, Wq, bq, Wk, bk, Wv, bv, Wo, bo):
    x = np.asarray(x, np.float32)
    Wq, bq = np.asarray(Wq, np.float32), np.asarray(bq, np.float32)
    Wk, bk = np.asarray(Wk, np.float32), np.asarray(bk, np.float32)
    Wv, bv = np.asarray(Wv, np.float32), np.asarray(bv, np.float32)
    Wo, bo = np.asarray(Wo, np.float32), np.asarray(bo, np.float32)

    nc = _get_nc()

    in_maps = []
    for c in range(NCORES):
        b, hg = divmod(c, 2)
        hs = slice(hg * HPC * HD, (hg + 1) * HPC * HD)
        woT = Wo[:, hs].T  # [384, 768]
        wo_pack = np.ascontiguousarray(
            woT.reshape(3, 128, EMB).transpose(1, 0, 2).reshape(128, 3 * EMB))
        wq_et = _etile_pack(_pad_headsT(Wq[hs])).reshape(128, NE, HPC, HDP)
        wk_et = _etile_pack(_pad_headsT(Wk[hs])).reshape(128, NE, HPC, HDP)
        in_maps.append({
            "xtp": _x_block_pack(x[b]),
            "wqp0": np.ascontiguousarray(
                wq_et[:, :, 0].reshape(128, NE * HDP)),
            "wqpr": np.ascontiguousarray(
                wq_et[:, :, 1:].reshape(128, NE * 3 * HDP)),
            "wkp0": np.ascontiguousarray(
                wk_et[:, :, 0].reshape(128, NE * HDP)),
            "wkpr": np.ascontiguousarray(
                wk_et[:, :, 1:].reshape(128, NE * 3 * HDP)),
            "wvp": _etile_pack(np.ascontiguousarray(Wv[hs].T)),
            "wop": wo_pack.astype(BF16),
            "bqp": _pad_bias(bq[hs]),
            "bkp": _pad_bias(bk[hs]),
            "identp": np.ascontiguousarray(np.eye(128, dtype=np.float32))
            .astype(BF16),
        })

    global LAST_RESULT
    trace = bool(int(os.environ.get("KERNEL_TRACE", "0")))
    tmpdir = os.environ.get("KERNEL_TRACE_DIR") or None
    res = run_bass_kernel_spmd(nc, in_maps, list(range(NCORES)), trace=trace,
                               tmpdir=tmpdir)
    LAST_RESULT = res

    out = np.empty((B, SEQ, EMB), np.float32)
    for b in range(B):
        out[b] = res.results[2 * b]["outp"] + res.results[2 * b + 1]["outp"]
    # bv enters each head's output additively (sum of softmax weights is 1),
    # and bo is a plain add: both fold into one constant vector.
    out += Wo @ bv + bo
    return out



# revision 90
# speedup vs baseline: 1.0818x; 1.0072x over previous
"""Multi-head attention (B=4, S=2048, E=768, H=8, D=96) on 8 Trainium2 cores.

Sharding: core c -> (batch b = c//2, head-group hg = c%2 of 4 heads).
Each core computes Q/K/V projections for its 4 heads over the full sequence
of its batch, full attention for those heads, and a partial output
projection (row-split Wo).  The two cores of a batch produce partial
outputs that are summed on the host during unsharding (tensor-parallel
reduce).

On-chip layout notes:
  - All matmul operands are bf16 (1 cycle/row on PE; fp32 would be 4x;
    fp8 DoubleRow would halve PE time but its ~3% per-element quantization
    noise lands ~1:1 in the output - the softmax average does not shrink
    relative error for zero-mean V - and busts the 2e-2 tolerance).
  - head_dim 96 is zero-padded to 128 (host pads Wq/Wk columns), so every
    matmul has K=128 contraction and 128-column stationary operands (FWL).
  - Attention scores are computed transposed, S^T[k, q] = K^T.T @ Q^T,
    so softmax normalization is a partition reduction; we get the sums for
    free by augmenting V with a ones column (row 96 of the O^T accumulator
    is then sum_k exp(S)).
  - exp runs on the scalar engine straight out of PSUM ([128,1024] over a
    pair of key tiles) with the 1/sqrt(d) scale folded into the
    activation's scale parameter.  PE is the bottleneck engine (~88%
    busy); ACT has slack, so it also absorbs some PSUM evictions.
  - Per-(head, q-chunk) normalization: sums row -> DRAM -> broadcast-DMA
    to 96 partitions -> fast reciprocal on DVE -> tensor_tensor mults.
    The two sums DMAs ride the Pool/SWDGE queue so they never head-of-line
    block the SP queue that carries input loads and output stores.  For
    the LAST head the roundtrip (~6 us) would stall the dependent
    output-projection chunks, so there the sums row is broadcast across
    96 partitions with a K=1 PE matmul against a ones column (~0.3 us),
    and the normalization multiplies are sliced per 128-column
    output-projection tile so each tile unblocks as early as possible.
  - Inputs are host-packed so each operand is ONE [128, n] DMA (the SP
    sequencer spends ~650 ns per DMA issue; 47 small input DMAs would
    serialize into a ~26 us head).  x is packed seq-block-major so the
    first K/Q chunk needs only wk + 0.75MB; biases load early because
    they gate the K/Q psum evictions; head 0's K/Q weight columns load
    before the other heads'.
  - PE stream order: head 0's K/Q projections first (they gate the ACT
    exp stream), V projection chunks interleaved per x block, then the
    attention pipeline; K/Q chunks of head h+1 are interleaved into head
    h's attention stream.  Output-projection chunks can only run
    during/after the last head (they need all four heads' softmax
    normalizations), but head 3's attnT rows live solely in packed tile
    2 - so each chunk splits into a t0/t1 partial (independent of the
    newest normalization, injected at the bare front pairs of the
    following q-chunk's stream) and a t2 finish (accumulation close +
    evictions + split stores) injected from pair 3.  The final q-chunk's
    four chunks run the same two-phase way in the tail, borrowing the
    then-idle attention psum rings so the partials overlap the last
    normalization chain.
"""

import os
import sys

sys.path.insert(0, "/opt/trn_rl_repo")

import numpy as np
import ml_dtypes

import concourse.bacc as bacc
import concourse.bass as bass
import concourse.tile as tile
from concourse import mybir
from concourse.bass_utils import run_bass_kernel_spmd

BF16 = ml_dtypes.bfloat16


def _pool_activation(nc, out, in_, func, scale=1.0):
    """Emit an InstActivation on the Pool/GPSIMD engine (bass only defines
    activation() on the scalar engine; the instruction itself is engine-
    tagged by the emitting queue, and the gpsimd DSP implements it)."""
    return bass.BassScalarEngine.activation(
        nc.gpsimd, out, in_, func, scale=scale)

EMB = 768
HEADS = 8
HD = 96          # true head dim
HDP = 128        # padded head dim
SEQ = 2048
B = 4
NCORES = 8
HPC = 4          # heads per core
SCALING = HD ** -0.5
QC = 512         # query chunk per attention inner loop
NQC = SEQ // QC
NKT = SEQ // 128  # 16 key tiles
NPAIR = NKT // 2
NE = EMB // 128   # 6 e_in tiles

_NC_CACHE = {}
LAST_RESULT = None  # BassKernelResults of the most recent run (for test.py)


def _build_nc():
    f32 = mybir.dt.float32
    bf = mybir.dt.bfloat16

    nc = bacc.Bacc(trn_type="TRN2", target_bir_lowering=False, debug=False,
                   num_devices=NCORES)

    # All operands host-packed into [128, n] so each loads as ONE DMA.
    xtp = nc.dram_tensor("xtp", [128, NE * SEQ], bf, kind="ExternalInput").ap()
    # K/Q weights split: head 0's columns load first (0.19MB) so head 0's
    # projections - which gate the whole pipeline - start ~5 us earlier
    wqp0 = nc.dram_tensor("wqp0", [128, NE * HDP], bf,
                          kind="ExternalInput").ap()
    wqpr = nc.dram_tensor("wqpr", [128, NE * 3 * HDP], bf,
                          kind="ExternalInput").ap()
    wkp0 = nc.dram_tensor("wkp0", [128, NE * HDP], bf,
                          kind="ExternalInput").ap()
    wkpr = nc.dram_tensor("wkpr", [128, NE * 3 * HDP], bf,
                          kind="ExternalInput").ap()
    wvp = nc.dram_tensor("wvp", [128, NE * HPC * HD], bf,
                         kind="ExternalInput").ap()
    wop = nc.dram_tensor("wop", [128, 3 * EMB], bf, kind="ExternalInput").ap()
    bqp = nc.dram_tensor("bqp", [128, HPC], f32, kind="ExternalInput").ap()
    bkp = nc.dram_tensor("bkp", [128, HPC], f32, kind="ExternalInput").ap()
    identp = nc.dram_tensor("identp", [128, 128], bf,
                            kind="ExternalInput").ap()
    outp = nc.dram_tensor("outp", [SEQ, EMB], f32, kind="ExternalOutput").ap()

    with tile.TileContext(nc) as tc:
        with (
            tc.tile_pool(name="const", bufs=1) as constp,
            tc.tile_pool(name="big", bufs=1) as bigp,
            tc.tile_pool(name="expp", bufs=6) as expp,
            tc.tile_pool(name="rbp", bufs=4) as rbp,
            tc.tile_pool(name="normp", bufs=1) as normp,
            tc.tile_pool(name="outsb", bufs=12) as outsb,
            tc.tile_pool(name="ps_proj", bufs=2, space="PSUM") as ps_proj,
            tc.tile_pool(name="ps_o", bufs=2, space="PSUM") as ps_o,
            tc.tile_pool(name="ps_pair", bufs=2, space="PSUM") as ps_pair,
        ):
            # ---- loads. x is packed seq-block-major ([128, 6e x 512] per
            # 512-sequence block) so the first K/Q chunk only needs wk + one
            # 0.75MB block; wk + block 0/1 load first, k/q chunks of head 0
            # then pipeline behind the remaining block DMAs. ----
            XB = NE * 512  # 3072 cols per seq block
            wk0_sb = constp.tile([128, NE * HDP], bf, name="wk0_sb")
            nc.sync.dma_start(out=wk0_sb, in_=wkp0)
            xtb = [bigp.tile([128, XB], bf, name=f"xtb{n}") for n in range(4)]
            # the first x half rides the Pool/SWDGE queue: its descriptor
            # path is independent of the HWDGE slot the weight loads use,
            # so it overlaps wk0 end-to-end
            nc.gpsimd.dma_start(out=xtb[0][:, 0:XB // 2],
                                in_=xtp[:, 0:XB // 2])
            nc.sync.dma_start(out=xtb[0][:, XB // 2:XB],
                              in_=xtp[:, XB // 2:XB])
            # biases next: tiny, but they gate the K/Q psum evictions
            bq_sb = constp.tile([128, HPC], f32, name="bq_sb")
            nc.sync.dma_start(out=bq_sb, in_=bqp)
            bk_sb = constp.tile([128, HPC], f32, name="bk_sb")
            nc.sync.dma_start(out=bk_sb, in_=bkp)
            wq0_sb = constp.tile([128, NE * HDP], bf, name="wq0_sb")
            nc.sync.dma_start(out=wq0_sb, in_=wqp0)
            wv_sb = constp.tile([128, NE * HPC * HD], bf, name="wv_sb")
            nc.sync.dma_start(out=wv_sb, in_=wvp)
            nc.sync.dma_start(out=xtb[1], in_=xtp[:, XB:2 * XB])
            wkr_sb = constp.tile([128, NE * 3 * HDP], bf, name="wkr_sb")
            nc.sync.dma_start(out=wkr_sb, in_=wkpr)
            wqr_sb = constp.tile([128, NE * 3 * HDP], bf, name="wqr_sb")
            nc.sync.dma_start(out=wqr_sb, in_=wqpr)
            nc.sync.dma_start(out=xtb[2], in_=xtp[:, 2 * XB:3 * XB])
            nc.sync.dma_start(out=xtb[3], in_=xtp[:, 3 * XB:4 * XB])
            wo_sb = constp.tile([128, 3 * EMB], bf, name="wo_sb")
            nc.sync.dma_start(out=wo_sb, in_=wop)
            # identity for the PE repack transposes (first needed in head
            # 3's phase, so it loads last)
            ident_sb = constp.tile([128, 128], bf, name="ident_sb")
            nc.sync.dma_start(out=ident_sb, in_=identp)

            def wk_eh(e, h):
                if h == 0:
                    return wk0_sb[:, e * HDP:(e + 1) * HDP]
                return wkr_sb[:, (e * 3 + h - 1) * HDP:(e * 3 + h) * HDP]

            def wq_eh(e, h):
                if h == 0:
                    return wq0_sb[:, e * HDP:(e + 1) * HDP]
                return wqr_sb[:, (e * 3 + h - 1) * HDP:(e * 3 + h) * HDP]

            def wv_e(e):
                return wv_sb[:, e * HPC * HD:(e + 1) * HPC * HD]

            def wo_t(t_):
                return wo_sb[:, t_ * EMB:(t_ + 1) * EMB]

            # ---- persistent intermediates ----
            # vaug: per key tile, 4 heads x (96 v-cols + a ones col).  The
            # ones col makes row^T @ vaug yield the softmax denominator in
            # the same accumulator (col 96 of each head group).  No pad
            # cols: vaug is the MOVING operand of the flipped O matmul, so
            # narrower means cheaper (97 vs 128 cycles).
            HDV = HD + 1
            vaug = []
            for kt in range(NKT):
                t = bigp.tile([128, HPC * HDV], bf, name=f"vaug{kt}")
                ones_cols = t.rearrange("p (h c) -> p h c",
                                        h=HPC)[:, :, HD:HD + 1]
                nc.gpsimd.memset(ones_cols, 1.0)
                vaug.append(t)
            qT = [bigp.tile([128, SEQ], bf, name=f"qT{h}") for h in range(HPC)]
            kT = [bigp.tile([128, SEQ], bf, name=f"kT{h}") for h in range(HPC)]
            # packed attention output, [384 rows = 3 tiles x 128, seq],
            # written by the h3-phase repack transposes
            attnT = [bigp.tile([128, SEQ], bf, name=f"attnT{t_}")
                     for t_ in range(3)]
            # normalized O in [q, head-dims] layout, per (q-chunk, q-tile):
            # written per head as its window completes, transposed into
            # attnT once all four heads are in
            normed = [[normp.tile([128, HPC * HD], bf, name=f"nm{qc}_{qt}")
                       for qt in range(4)] for qc in range(NQC)]

            f32_ = f32

            # ---- projection emit helpers ----
            def emit_v_chunk(kt):
                psv = ps_proj.tile([128, 512], f32_, tag="ps",
                                   name=f"psv{kt}")
                blk, off = divmod(kt, 4)
                for e in range(NE):
                    nc.tensor.matmul(psv[:, 0:HPC * HD],
                                     lhsT=xtb[blk][:, e * 512 + off * 128:
                                                   e * 512 + off * 128 + 128],
                                     rhs=wv_e(e),
                                     start=(e == 0), stop=(e == NE - 1))
                for hh in range(HPC):
                    nc.vector.tensor_copy(
                        vaug[kt][:, hh * HDV:hh * HDV + HD],
                        psv[:, hh * HD:(hh + 1) * HD])

            kq_ps = {}

            def emit_kq_part(h, n, which, part):
                """One half of a K/Q chunk (3 of 6 e-matmuls).  Thunking
                chunks at this granularity keeps the per-pair PE load above
                the ACT exp cadence, so the S stream never outruns ACT into
                the 2-slot pss ring."""
                key = (h, n, which)
                w_eh, dst, b_sb = ((wk_eh, kT, bk_sb) if which == "k"
                                   else (wq_eh, qT, bq_sb))
                if part == 0:
                    kq_ps[key] = ps_proj.tile([128, 512], f32_, tag="ps",
                                              name=f"ps{which}{h}_{n}")
                ps = kq_ps[key]
                for e in (3 * part, 3 * part + 1, 3 * part + 2):
                    nc.tensor.matmul(ps,
                                     lhsT=w_eh(e, h),
                                     rhs=xtb[n][:, e * 512:(e + 1) * 512],
                                     start=(e == 0), stop=(e == NE - 1))
                if part == 1:
                    nsl = slice(n * 512, (n + 1) * 512)
                    nc.vector.tensor_scalar_add(dst[h][:, nsl],
                                                kq_ps.pop(key),
                                                b_sb[:, h:h + 1])

            def emit_kq_chunk(h, n, which):
                for part in range(2):
                    emit_kq_part(h, n, which, part)

            def kq_chunks(h):
                for n in range(4):
                    yield ("k", h, n)
                    yield ("q", h, n)

            # ---- output projection chunk (one 128-row q tile) ----
            # Split across two 1-bank psums so it can borrow ps_proj slots;
            # PSUM->SBUF copies go on DVE (ACT is busy with exp here).
            # Tail out-proj chunks, two-phase: head 3's rows live only in
            # packed attnT tile 2, so the t0/t1 matmuls are independent of
            # the final normalization and run DURING its chain; only the t2
            # matmul (accumulation close) waits for the per-qm norm slice.
            # Psums borrow the attention rings (free by then).
            tail_ps = {}

            def emit_out_partial(qm, mode="tail"):
                qsl = slice(qm * 128, (qm + 1) * 128)
                if mode == "win":
                    psA = ps_proj.tile([128, 512], f32_, tag="ps",
                                       name=f"poA{qm}")
                    psB = ps_proj.tile([128, 512], f32_, tag="ps",
                                       name=f"poB{qm}")
                elif mode == "edge":
                    # last-window partial: psA on the ps ring, psB on the
                    # pso ring's spare slot (the previous psof freed once
                    # its normalization drained) so only ONE ps slot is
                    # held across the window/tail boundary
                    psA = ps_proj.tile([128, 512], f32_, tag="ps",
                                       name=f"poA{qm}")
                    psB = ps_o.tile([128, 512], f32_, tag="pso",
                                    name=f"poB{qm}")
                else:
                    # tail: both halves on the pss ring (free then)
                    psA = ps_pair.tile([128, 512], f32_, tag="pss",
                                       name=f"poA{qm}")
                    psB = ps_pair.tile([128, 512], f32_, tag="pss",
                                       name=f"poB{qm}")
                tail_ps[qm] = (psA, psB)
                for t in range(2):
                    nc.tensor.matmul(psA,
                                     lhsT=attnT[t][:, qsl],
                                     rhs=wo_t(t)[:, 0:512],
                                     start=(t == 0), stop=False)
                    nc.tensor.matmul(psB[:, 0:256],
                                     lhsT=attnT[t][:, qsl],
                                     rhs=wo_t(t)[:, 512:768],
                                     start=(t == 0), stop=False)

            def emit_out_finish(qm, windowed=False):
                qsl = slice(qm * 128, (qm + 1) * 128)
                psA, psB = tail_ps.pop(qm)
                nc.tensor.matmul(psA, lhsT=attnT[2][:, qsl],
                                 rhs=wo_t(2)[:, 0:512],
                                 start=False, stop=True)
                nc.tensor.matmul(psB[:, 0:256], lhsT=attnT[2][:, qsl],
                                 rhs=wo_t(2)[:, 512:768],
                                 start=False, stop=True)
                out_sb = outsb.tile([128, EMB], f32_, tag="osb",
                                    name=f"osb{qm}")
                # evictions on DVE in windows (ACT is exp-saturated there;
                # GPSIMD cannot touch PSUM on real hw).  In the tail, psB
                # rides ACT end-to-end (evict + store) so the last stores
                # split across the SP and ACT queues instead of
                # serializing on SP's 650 ns issue cost.
                nc.vector.tensor_copy(out_sb[:, 0:512], psA)
                if windowed:
                    nc.sync.dma_start(
                        out=outp[qm * 128:(qm + 1) * 128, 0:512],
                        in_=out_sb[:, 0:512])
                else:
                    eng = nc.gpsimd if qm % 2 == 0 else nc.sync
                    eng.dma_start(
                        out=outp[qm * 128:(qm + 1) * 128, 0:512],
                        in_=out_sb[:, 0:512])
                if windowed:
                    nc.vector.tensor_copy(out_sb[:, 512:768], psB[:, 0:256])
                    nc.sync.dma_start(
                        out=outp[qm * 128:(qm + 1) * 128, 512:768],
                        in_=out_sb[:, 512:768])
                else:
                    # tail: psB evict on DVE; store rotated across the
                    # ACT and Pool/SWDGE queues so the four final stores
                    # don't serialize on the single HWDGE slot
                    nc.vector.tensor_copy(out_sb[:, 512:768], psB[:, 0:256])
                    eng = nc.scalar if qm % 2 == 0 else nc.gpsimd
                    eng.dma_start(
                        out=outp[qm * 128:(qm + 1) * 128, 512:768],
                        in_=out_sb[:, 512:768])

            # ---- attention emit (with interleaved PE filler work) ----
            # repack: transpose normed[qc][qt] ([128 q, 384 packed head
            # dims]) into attnT via 128x128 PE transposes + DVE evictions.
            # Chunks t0/t1 cover only heads 0-2, so they repack as soon as
            # head 2's normalization lands - during head 3's own windows -
            # and the out-proj partials (which read t0/t1) can then run
            # inside the last window.  Only the t2 chunk (heads 2+3) waits
            # for head 3's normalization.
            def emit_repack01(qc, qt):
                tps = ps_proj.tile([128, 2 * 128], bf, tag="ps",
                                   name=f"tp01_{qc}_{qt}")
                for t_ in range(2):
                    nc.tensor.transpose(
                        tps[:, t_ * 128:(t_ + 1) * 128],
                        normed[qc][qt][:, t_ * 128:(t_ + 1) * 128],
                        ident_sb)
                csl = slice(qc * QC + qt * 128, qc * QC + (qt + 1) * 128)
                for t_ in range(2):
                    nc.vector.tensor_copy(attnT[t_][:, csl],
                                          tps[:, t_ * 128:(t_ + 1) * 128])

            def emit_repack2(qc, qt, tail=False):
                tps = ps_proj.tile([128, 128], bf, tag="ps",
                                   name=f"tp2_{qc}_{qt}")
                nc.tensor.transpose(tps, normed[qc][qt][:, 256:384],
                                    ident_sb)
                csl = slice(qc * QC + qt * 128, qc * QC + (qt + 1) * 128)
                if tail:
                    # keep the tail's DVE queue clear for the out evictions
                    nc.scalar.activation(attnT[2][:, csl], tps,
                                         mybir.ActivationFunctionType.Copy)
                else:
                    nc.vector.tensor_copy(attnT[2][:, csl], tps)

            # cross-window O backlog: each pair's O-block (and, for a
            # window's last pair, its normalization) is queued and drained
            # `lag` pairs later - ACROSS window boundaries for heads 0-2,
            # so a window's trailing O's overlap the next window's S
            # stream.  Head-3 windows drain fully at their end because the
            # following window's repack thunks need the normalization.
            obl = []

            def emit_attention(h, thunks_for_qc):
                """thunks_for_qc(qc) -> list of emit callables injected into
                the PE stream spread across this q-chunk's pairs."""
                for qc in range(NQC):
                    thunks, pos = thunks_for_qc(qc)
                    inject_at = {}
                    if pos is not None:
                        for i, t in enumerate(thunks):
                            inject_at.setdefault(
                                pos[i] if i < len(pos) else NPAIR - 1,
                                []).append(t)
                    else:
                        step = max(NPAIR // max(len(thunks), 1), 1)
                        for i, t in enumerate(thunks):
                            inject_at.setdefault(
                                min(1 + i * step, NPAIR - 1), []).append(t)
                    qsl = slice(qc * QC, (qc + 1) * QC)
                    idx = h * NQC + qc
                    # flipped-O accumulator: [128 q, 4 qt x (96 d + sums)]
                    # packed per q-tile into one psum bank.  The bank holds
                    # FOUR interleaved accumulation chains; hardware supports
                    # only one OPEN start/stop group per bank at a time
                    # (later start=True resets the others), so the tile is
                    # zeroed up front and every matmul accumulates with
                    # start=False.
                    psof = ps_o.tile([128, 4 * HDV], f32_, tag="pso",
                                     name=f"psof{idx}")
                    nc.vector.memset(psof, 0.0)
                    eps = []

                    def emit_ss(p):
                        pss = ps_pair.tile([128, 1024], f32_, tag="pss",
                                           name=f"pss{idx}_{p}")
                        ep = expp.tile([128, 1024], bf, tag="exp",
                                       name=f"exp{idx}_{p}")
                        # exp split: Pool takes the FIRST cols (waits only
                        # on the j=0 matmul, so its q7-launch latency hides
                        # behind the j=1 matmul), ACT takes the rest.  Both
                        # cadences stay under the pair's PE work so PE is
                        # the limiter.  Head 3 (qc>0) keeps exp fully on ACT
                        # - its windows carry out-proj thunks (PE/pair well
                        # above 1038 ns) and Pool absorbs psum evictions.
                        # both S matmuls BEFORE either exp: an exp emitted
                        # between them reads this pss tile and Tile's
                        # conservative subtile tracking then serializes the
                        # second matmul behind it (false intra-tile WAR)
                        nc.tensor.matmul(
                            pss[:, 0:512],
                            lhsT=kT[h][:, (2 * p) * 128:(2 * p + 1) * 128],
                            rhs=qT[h][:, qsl],
                            start=True, stop=True)
                        nc.tensor.matmul(
                            pss[:, 512:1024],
                            lhsT=kT[h][:, (2 * p + 1) * 128:
                                       (2 * p + 2) * 128],
                            rhs=qT[h][:, qsl],
                            start=True, stop=True)
                        nc.scalar.activation(
                            ep, pss,
                            mybir.ActivationFunctionType.Exp,
                            scale=SCALING)
                        eps.append(ep)

                    def emit_o(p, ep, psof=psof, h=h):
                        # flipped: the exp tile is the (free) stationary
                        # load, the 97-col vaug slab streams -> 40 ns per
                        # matmul instead of 213, output lands [q, d]
                        for j in range(2):
                            kt = 2 * p + j
                            for qt in range(4):
                                nc.tensor.matmul(
                                    psof[:, qt * HDV:(qt + 1) * HDV],
                                    lhsT=ep[:, j * 512 + qt * 128:
                                            j * 512 + (qt + 1) * 128],
                                    rhs=vaug[kt][:, h * HDV:(h + 1) * HDV],
                                    start=False, stop=(kt == NKT - 1))

                    def emit_norm(psof=psof, h=h, qc=qc, idx=idx):
                        # sums live at col 96 of each q-tile group; the
                        # reciprocal is a per-partition scalar.  One
                        # contiguous single-column reciprocal per q-tile
                        # (the custom-DVE op mis-lowers strided APs).
                        rbq = rbp.tile([128, 4], f32_, tag="rbq",
                                       name=f"rbq{idx}")
                        for qt in range(4):
                            nc.vector.reciprocal_approx_fast(
                                out=rbq[:, qt:qt + 1],
                                in_=psof[:, qt * HDV + HD:qt * HDV + HD + 1])
                        for qt in range(4):
                            nc.vector.tensor_scalar_mul(
                                normed[qc][qt][:, h * HD:(h + 1) * HD],
                                psof[:, qt * HDV:qt * HDV + HD],
                                rbq[:, qt:qt + 1])

                    def make_o(p, last, ep):
                        def f():
                            emit_o(p, ep)
                            if last:
                                emit_norm()
                        return f

                    lag = 4
                    for p in range(NPAIR):
                        emit_ss(p)
                        for t in inject_at.get(p, ()):
                            t()
                        obl.append(make_o(p, p == NPAIR - 1, eps[-1]))
                        while len(obl) > lag:
                            obl.pop(0)()
                    while obl:
                        obl.pop(0)()

            # ---- emission schedule ----
            # consume each x seq-block as its DMA lands: head 0's k/q chunks
            # for block n, then the V chunks of block n
            for n in range(4):
                emit_kq_chunk(0, n, "k")
                emit_kq_chunk(0, n, "q")
                for kt in range(4 * n, 4 * n + 4):
                    emit_v_chunk(kt)

            def kq_thunks(hnext):
                def f(qc):
                    # 2 chunks per q-chunk, split into 3-matmul halves (4
                    # thunks of ~640 ns) so the PE filler spreads across the
                    # window instead of lumping at two pairs
                    items = list(kq_chunks(hnext))[2 * qc:2 * qc + 2]
                    out = []
                    for it in items:
                        for part in range(2):
                            out.append(
                                lambda it=it, part=part: emit_kq_part(
                                    it[1], it[2], it[0], part))
                    return out, None
                return f

            def rp01_thunks(qc):
                return [lambda qt=qt: emit_repack01(qc, qt)
                        for qt in range(4)]

            def out_thunks(qc):
                # head-3 window qc: close out q-chunk qc-1 (t2 repack + P/F
                # two-phase out-proj) and pre-repack t0/t1 of q-chunk qc+1
                # (they only need heads 0-2, normalized long ago).  The
                # last window also starts P(q0+0) with its psB on the pso
                # spare slot so the tail begins with a partial in flight.
                if qc == 0:
                    return (rp01_thunks(0) + rp01_thunks(1)
                            + rp01_thunks(2),
                            [0, 1, 1, 2, 3, 3, 4, 5, 5, 6, 7, 7])
                qp = qc - 1
                out = []
                for qt4 in range(0, 4, 2):
                    out.append(lambda qt=qt4: emit_repack2(qp, qt))
                    out.append(lambda qt=qt4 + 1: emit_repack2(qp, qt))
                    for qm in (4 * qp + qt4, 4 * qp + qt4 + 1):
                        out.append(
                            lambda qm=qm: emit_out_partial(qm, "win"))
                        out.append(lambda qm=qm: emit_out_finish(qm, True))
                pos = [0, 0, 1, 1, 2, 2, 3, 3, 4, 4, 5, 5]
                if qc == NQC - 2:
                    out += rp01_thunks(qc + 1)
                    pos += [6, 6, 7, 7]
                elif qc == NQC - 1:
                    out.append(lambda: emit_out_partial(4 * qc, "edge"))
                    pos += [7]
                return out, pos

            for h in range(HPC - 1):
                emit_attention(h, kq_thunks(h + 1))
            emit_attention(HPC - 1, out_thunks)
            # tail: only the t2 repacks and the finish halves remain; each
            # F(qm) frees the ps/pso slots its successor partial needs
            q0 = 4 * (NQC - 1)
            emit_repack2(NQC - 1, 0, True)
            emit_out_finish(q0)
            emit_out_partial(q0 + 1)
            emit_repack2(NQC - 1, 1, True)
            emit_out_finish(q0 + 1)
            emit_out_partial(q0 + 2)
            emit_repack2(NQC - 1, 2, True)
            emit_out_finish(q0 + 2)
            emit_out_partial(q0 + 3)
            emit_repack2(NQC - 1, 3, True)
            emit_out_finish(q0 + 3)

            if os.environ.get("KDEBUG"):
                dbg_attn = nc.dram_tensor(
                    "dbg_attn", [3 * 128, SEQ], bf,
                    kind="ExternalOutput").ap()
                for t_ in range(3):
                    nc.sync.dma_start(
                        out=dbg_attn[t_ * 128:(t_ + 1) * 128, :],
                        in_=attnT[t_])
                dbg_nm = nc.dram_tensor(
                    "dbg_nm", [NQC * 4 * 128, HPC * HD], bf,
                    kind="ExternalOutput").ap()
                for qc in range(NQC):
                    for qt in range(4):
                        r0 = (qc * 4 + qt) * 128
                        nc.sync.dma_start(out=dbg_nm[r0:r0 + 128, :],
                                          in_=normed[qc][qt])

    nc.compile()
    return nc


def _get_nc():
    if "nc" not in _NC_CACHE:
        _NC_CACHE["nc"] = _build_nc()
    return _NC_CACHE["nc"]


def _etile_pack(wT):
    """[768, n] (e on rows) -> [128, 6*n] bf16: e-tiles as column blocks so
    the whole operand loads as ONE [128, n] DMA."""
    n = wT.shape[1]
    a = wT.reshape(NE, 128, n).transpose(1, 0, 2)
    return np.ascontiguousarray(a.reshape(128, NE * n)).astype(BF16)


def _x_block_pack(x_b):
    """[2048, 768] x -> [128, 4 * 6 * 512] bf16, seq-block-major: block n
    holds e-tiles of sequence rows n*512..(n+1)*512 as column slabs."""
    a = x_b.reshape(4, 512, NE, 128)          # n, s, e, p
    a = a.transpose(3, 0, 2, 1)               # p, n, e, s
    return np.ascontiguousarray(a.reshape(128, 4 * NE * 512)).astype(BF16)


def _pad_headsT(w_rows):
    """[384, 768] head rows -> zero-pad head dim 96->128 -> transpose -> [768, 512]."""
    p = np.zeros((HPC * HDP, EMB), np.float32)
    p.reshape(HPC, HDP, EMB)[:, :HD] = w_rows.reshape(HPC, HD, EMB)
    return np.ascontiguousarray(p.T)


def _pad_bias(b_rows):
    """[384] head bias -> [128, HPC] padded/transposed for per-partition add."""
    p = np.zeros((HPC, HDP), np.float32)
    p[:, :HD] = b_rows.reshape(HPC, HD)
    return np.ascontiguousarray(p.T)


def kernel(x, Wq, bq, Wk, bk, Wv, bv, Wo, bo):
    x = np.asarray(x, np.float32)
    Wq, bq = np.asarray(Wq, np.float32), np.asarray(bq, np.float32)
    Wk, bk = np.asarray(Wk, np.float32), np.asarray(bk, np.float32)
    Wv, bv = np.asarray(Wv, np.float32), np.asarray(bv, np.float32)
    Wo, bo = np.asarray(Wo, np.float32), np.asarray(bo, np.float32)

    nc = _get_nc()

    in_maps = []
    for c in range(NCORES):
        b, hg = divmod(c, 2)
        hs = slice(hg * HPC * HD, (hg + 1) * HPC * HD)
        woT = Wo[:, hs].T  # [384, 768]
        wo_pack = np.ascontiguousarray(
            woT.reshape(3, 128, EMB).transpose(1, 0, 2).reshape(128, 3 * EMB))
        wq_et = _etile_pack(_pad_headsT(Wq[hs])).reshape(128, NE, HPC, HDP)
        wk_et = _etile_pack(_pad_headsT(Wk[hs])).reshape(128, NE, HPC, HDP)
        in_maps.append({
            "xtp": _x_block_pack(x[b]),
            "wqp0": np.ascontiguousarray(
                wq_et[:, :, 0].reshape(128, NE * HDP)),
            "wqpr": np.ascontiguousarray(
                wq_et[:, :, 1:].reshape(128, NE * 3 * HDP)),
            "wkp0": np.ascontiguousarray(
                wk_et[:, :, 0].reshape(128, NE * HDP)),
            "wkpr": np.ascontiguousarray(
                wk_et[:, :, 1:].reshape(128, NE * 3 * HDP)),
            "wvp": _etile_pack(np.ascontiguousarray(Wv[hs].T)),
            "wop": wo_pack.astype(BF16),
            "bqp": _pad_bias(bq[hs]),
            "bkp": _pad_bias(bk[hs]),
            "identp": np.ascontiguousarray(np.eye(128, dtype=np.float32))
            .astype(BF16),
        })

    global LAST_RESULT
    trace = bool(int(os.environ.get("KERNEL_TRACE", "0")))
    tmpdir = os.environ.get("KERNEL_TRACE_DIR") or None
    res = run_bass_kernel_spmd(nc, in_maps, list(range(NCORES)), trace=trace,
                               tmpdir=tmpdir)
    LAST_RESULT = res

    out = np.empty((B, SEQ, EMB), np.float32)
    for b in range(B):
        out[b] = res.results[2 * b]["outp"] + res.results[2 * b + 1]["outp"]
    # bv enters each head's output additively (sum of softmax weights is 1),
    # and bo is a plain add: both fold into one constant vector.
    out += Wo @ bv + bo
    return out

